# revision 37
# baseline (speedup 1.0000x reference)
"""GatedLinearAttentionARMA on 8 Trainium2 NeuronCores (Bass/Tile).

B=2, L=512, D=1024, H=16, DH=64.

Both recurrences are linear scans of rank-1 updates, exactly equivalent to
causal quadratic attention:
  O1_t = Gc_t * sum_{s<=t} (Q_t.K_s) * u_s * V_s        (u = silu(K@sw)/Gc)
  E_t  = V_{t+1} - O1_t
  O2_t = sum_{s<t} (q2_{t-1}.k2_s) * E_s
  y    = (O1 + O2) @ cp_w + cp_b

Sharding: core c handles batch b = c//4 and heads (c%4)*4 .. +3.  Each core
emits its heads' d-major partial output yT (1024, 512); the host sums the 4
per-batch partials, transposes, and adds cp_b.

v2 device program (vs the 47.5us baseline):
  - all matmul operands shipped/evacuated as bf16 (PSUM accumulation stays
    f32): halves DMA bytes and makes <256-col matmuls 4x faster on PE.
  - inputs ride 3 parallel DMA queues (SP / Act / Pool) ordered so the first
    projection can start ~3.5us in; outputs are spread over all 3 queues.
  - activation-table warmup at t~0 on a memset tile (the 1.28us table load
    otherwise lands in the first projection evacuation).
  - PSUM evacuations balanced across Act/DVE; masks and m1 on Pool.
"""

import sys

if "/opt/trn_rl_repo" not in sys.path:
    sys.path.insert(0, "/opt/trn_rl_repo")

import numpy as np
import ml_dtypes

B, L, D, H = 2, 512, 1024, 16
DH = D // H            # 64
NCH = L // 128         # 4 time chunks
KC = D // 128          # 8 contraction chunks
N_CORES = 8

# --- packed input column layout (fp32-column units) -----------------------
# dtype 'b' = bf16 (2 elems per fp32 column), 'f' = fp32
_REG = {}
_c = 0
def _alloc(name, cols_f32, dt):
    global _c
    _REG[name] = (_c, _c + cols_f32, dt)
    _c += cols_f32
# Act queue: wq + masks + idn + scalars (needed first by PE / Pool masks)
_alloc("wq", KC * 128, 'b')         # (128, 8, 256) bf16
_alloc("mi", 64, 'b')               # (128, 128) incl causal mask
_alloc("ms8", 64, 'b')              # strict causal mask
_alloc("idn", 32, 'b')              # (64, 64) identity for PE transpose
_alloc("u", 16, 'f')                # (128, 4, 4)
_alloc("gc", 16, 'f')
_alloc("bq", 2, 'f')
_alloc("bk", 2, 'f')
_alloc("bqr", 2, 'f')               # 0.98*bq
_alloc("bk2s", 2, 'f')              # c2*k2_b
R_ACT_END = _c
_alloc("wcp", 2 * D // 2, 'b')      # (128, 2, 1024) bf16  (Act queue tail)
R_ACT2_END = _c
# SP queue: xT (split in 4), xT8 (fp8 copy for the DoubleRow k2 projection)
_alloc("xT", KC * 256, 'b')         # (128, 8, 512) bf16
_alloc("xT8", KC * 128, 'x')        # (128, 4, 2, 512) fp8e4
R_SP_END = _c
# Pool queue: wk, wk28, v, vs, gcp
_alloc("wk", KC * 128, 'b')
_alloc("wk28", KC * 64, 'x')        # (128, 4, 2, 256) fp8e4
_alloc("v", 4 * NCH * DH // 2, 'b')     # (128, 4, 4, 64) bf16
_alloc("vs", 4 * NCH * DH // 2, 'b')
_alloc("gcp", 2 * L // 2, 'b')          # (128p, 2, 512) bf16: Gc per (row-head, hp)
TOT = _c

_CACHE = {}


def _build_nc(split_waits=True):
    import concourse.bass as bass
    import concourse.mybir as mybir
    from concourse.tile import TileContext

    f32 = mybir.dt.float32
    f32r = mybir.dt.float32r
    bf16 = mybir.dt.bfloat16
    nc = bass.Bass()

    inp = nc.dram_tensor("inp", [128, TOT], f32r, kind="ExternalInput")
    yT = nc.dram_tensor("yT", [D, L], bf16, kind="ExternalOutput")

    C2 = 0.02 / float(np.sqrt(D))
    Ident = mybir.ActivationFunctionType.Identity
    Sig = mybir.ActivationFunctionType.Sigmoid
    Rel = mybir.ActivationFunctionType.Relu
    Cpy = mybir.ActivationFunctionType.Copy
    MUL = mybir.AluOpType.mult

    with TileContext(nc) as tc:
        with (
            tc.tile_pool(name="cst", bufs=1) as cst,
            tc.tile_pool(name="qk", bufs=1) as qk,
            tc.tile_pool(name="att", bufs=8) as att,
            tc.tile_pool(name="sm", bufs=4) as sm,
            tc.tile_pool(name="psA", bufs=2, space="PSUM") as psA,
            tc.tile_pool(name="psD", bufs=3, space="PSUM") as psD,
            tc.tile_pool(name="pot", bufs=2, space="PSUM") as pot,
        ):
            mega = cst.tile([128, TOT], f32r)

            # --- input DMAs: 3 parallel queues, first-needed-first --------
            # SP: xT in 4 pieces (drip-feeds the kc-interleaved projections),
            # then wcp.  Act: wq in 2 pieces, then masks+scalars, then free.
            # Pool: wk, wk2, v+vs+gcb.
            a0 = _REG["xT"][0]
            a8 = _REG["xT8"][0]
            for i in range(2):
                nc.sync.dma_start(out=mega[:, a0 + 512 * i:a0 + 512 * (i + 1)],
                                  in_=inp[:, a0 + 512 * i:a0 + 512 * (i + 1)])
            nc.sync.dma_start(out=mega[:, a8:R_SP_END], in_=inp[:, a8:R_SP_END])
            nc.sync.dma_start(out=mega[:, _REG["wcp"][0]:R_ACT2_END],
                              in_=inp[:, _REG["wcp"][0]:R_ACT2_END])
            nc.scalar.dma_start(out=mega[:, 0:128], in_=inp[:, 0:128])
            nc.scalar.dma_start(out=mega[:, 128:1024], in_=inp[:, 128:1024])
            for i in range(2, 4):
                nc.scalar.dma_start(out=mega[:, a0 + 512 * i:a0 + 512 * (i + 1)],
                                    in_=inp[:, a0 + 512 * i:a0 + 512 * (i + 1)])
            nc.scalar.dma_start(out=mega[:, 1024:R_ACT_END],
                                in_=inp[:, 1024:R_ACT_END])
            b0, b1 = _REG["wk"][0], _REG["wk28"][0]
            b2 = _REG["v"][0]
            nc.gpsimd.dma_start(out=mega[:, b0:b1], in_=inp[:, b0:b1])
            nc.gpsimd.dma_start(out=mega[:, b1:b2], in_=inp[:, b1:b2])
            nc.gpsimd.dma_start(out=mega[:, b2:TOT], in_=inp[:, b2:TOT])

            def bview(name, rows, *shape):
                a, b, dt = _REG[name]
                assert dt == 'b'
                ap = mega[0:rows, a:b].bitcast(bf16)
                if len(shape) > 1:
                    ap = ap.rearrange(
                        "p (" + " ".join(f"d{i}" for i in range(len(shape))) + ") -> p "
                        + " ".join(f"d{i}" for i in range(len(shape))),
                        **{f"d{i}": s for i, s in enumerate(shape)},
                    )
                return ap

            fp8 = mybir.dt.float8e4

            def xview(name, *shape):
                a, b, dt_ = _REG[name]
                ap = mega[:, a:b].bitcast(fp8)
                return ap.rearrange(
                    "p (" + " ".join(f"d{i}" for i in range(len(shape))) + ") -> p "
                    + " ".join(f"d{i}" for i in range(len(shape))),
                    **{f"d{i}": s for i, s in enumerate(shape)},
                )

            v_xT = bview("xT", 128, KC, L)
            v_xT8 = xview("xT8", KC // 2, 2, L)
            v_wk28 = xview("wk28", KC // 2, 2, 256)
            v_wq = bview("wq", 128, KC, 256)
            v_wk = bview("wk", 128, KC, 256)
            v_wcp = bview("wcp", 128, 2, D)
            v_v = bview("v", 128, 4, NCH, DH)
            v_vs = bview("vs", 128, 4, NCH, DH)
            v_gcp = bview("gcp", 128, 2, L)
            v_mi = bview("mi", 128, 128)
            v_ms8 = bview("ms8", 128, 128)
            v_idn = bview("idn", 64, 64)

            # --- act-table warmup at t~0 (Sigmoid loads the shared table) -
            wrm = sm.tile([1, 2], f32, tag="wrm", bufs=1)
            nc.gpsimd.memset(wrm[:], 0.25)
            nc.scalar.activation(wrm[0:1, 1:2], wrm[0:1, 0:1], Sig, scale=1.0)

            # --- PE p-state warmup: ~2us of dummy matmuls on a memset tile
            # (PE ramps 0.65->1.2->2.4GHz over 3us of continuous activity;
            # burn the ramp before the first weights arrive).
            grb = cst.tile([128, 256], bf16)
            nc.vector.memset(grb[:], 0.125)
            pdum = psD.tile([128, 256], f32, tag="ptr", bufs=1, name="pdum")
            NDUM = 9
            for i in range(NDUM):
                nc.tensor.matmul(pdum[:], grb[:, 0:128], grb[:, 0:256],
                                 start=(i == 0), stop=(i == NDUM - 1))

            # f32 copies of the per-partition scalar block (tensor_scalar and
            # activation bias operands must be plain float32)
            sc = cst.tile([128, 32], f32)    # u(16) + gc(16)
            nc.vector.tensor_copy(sc[:], mega[:, _REG["u"][0]:_REG["gc"][1]])
            scA = cst.tile([128, 8], f32)    # bq bk bqr bk2s
            nc.scalar.copy(scA[:], mega[:, _REG["bq"][0]:_REG["bk2s"][1]])
            v_u = sc[:, 0:16].rearrange("p (h c) -> p h c", h=4)
            v_gc = sc[:, 16:32].rearrange("p (h c) -> p h c", h=4)
            v_bq = scA[:, 0:2]
            v_bk = scA[:, 2:4]
            v_bqr = scA[:, 4:6]
            v_bk2s = scA[:, 6:8]

            def MM(out, lhsT, rhs, **kw):
                return nc.tensor.matmul(out, lhsT, rhs, **kw)

            # ---- projections: d-major QT/KT/k2T (2 heads per tile) ------
            QT = qk.tile([128, 2, L], bf16)
            QTG = qk.tile([128, 2, L], bf16)   # QT * Gc_t (per row-head)
            KT = qk.tile([128, 2, L], bf16)
            K2T = qk.tile([128, 2, L], bf16)
            Q2M = qk.tile([128, 2, L], bf16)   # min(Q, 0.02Q)
            Q2S = qk.tile([128, 2, L], bf16)   # Q2M shifted right by one

            def emit_proj(hp):
                # q and k matmuls interleaved kc-major so the xT chunks are
                # consumed as they arrive off the drip-fed SP queue.
                mcol = slice(hp * 128, (hp + 1) * 128)
                pq = psA.tile([128, L], f32, tag="pbig", name=f"pq_{hp}")
                pk = psA.tile([128, L], f32, tag="pbig", name=f"pk_{hp}")
                for kc in range(KC):
                    MM(pq[:], v_wq[:, kc, mcol], v_xT[:, kc, :],
                       start=(kc == 0), stop=(kc == KC - 1))
                nc.scalar.activation(QT[:, hp, :], pq[:], Ident,
                                     bias=v_bq[:, hp:hp + 1], scale=1.0)
                for kc in range(KC):
                    MM(pk[:], v_wk[:, kc, mcol], v_xT[:, kc, :],
                       start=(kc == 0), stop=(kc == KC - 1))
                nc.scalar.activation(KT[:, hp, :], pk[:], Ident,
                                     bias=v_bk[:, hp:hp + 1], scale=1.0)
                # branch-1 Q carries the gate: qtg_t = Q_t * Gc_t
                nc.vector.tensor_tensor(QTG[:, hp, :], QT[:, hp, :],
                                        v_gcp[:, hp, :], MUL)

                pk2 = pot.tile([128, L], f32, tag="pot", name=f"pk2_{hp}")
                for kc2 in range(KC // 2):
                    MM(pk2[:], v_wk28[:, kc2, :, mcol], v_xT8[:, kc2, :, :],
                       start=(kc2 == 0), stop=(kc2 == KC // 2 - 1),
                       perf_mode=mybir.MatmulPerfMode.DoubleRow)
                nc.scalar.activation(K2T[:, hp, :], pk2[:], Sig,
                                     bias=v_bk2s[:, hp:hp + 1], scale=C2)

                # q2m = min(Q, 0.02*Q)   (true q2 * 8)
                Q02 = sm.tile([128, L], bf16, tag="q02", bufs=2,
                              name=f"q02_{hp}")
                nc.vector.tensor_scalar(Q02[:], QT[:, hp, :], 0.02, None, MUL)
                nc.vector.tensor_tensor(Q2M[:, hp, :], QT[:, hp, :], Q02[:],
                                        mybir.AluOpType.min)
                nc.vector.tensor_copy(Q2S[:, hp, 1:L], Q2M[:, hp, 0:L - 1])
                nc.vector.tensor_scalar(Q2S[:, hp, 0:1], QT[:, hp, 0:1],
                                        0.0, None, MUL)

            # ---- per-head attention ------------------------------------
            outT2 = att.tile([128, 2, L], bf16, tag="otp", bufs=1)

            def emit_head(h):
                hp, r0 = h // 2, (h % 2) * 64
                rows = slice(r0, r0 + 64)
                qt = QTG[rows, hp, :]
                kt = KT[rows, hp, :]
                k2t = K2T[rows, hp, :]
                q2s = Q2S[rows, hp, :]

                # branch 1: S^T chunks -> A1 (u-scaled, causal incl).
                # cs=2 and cs=3 share one PSUM bank (column-packed).
                pa = psD.tile([128, L], f32, tag="pd", name=f"pa1_{h}")
                pb = psD.tile([128, L], f32, tag="pd", name=f"pb1_{h}")
                pc = psD.tile([128, L], f32, tag="pd", name=f"pc1_{h}")
                s_plan = [(0, pa, 0), (1, pb, 0), (2, pc, 0), (3, pc, 256)]
                for cs, ps1, cb in s_plan:
                    c0 = cs * 128
                    MM(ps1[:, cb:cb + L - c0], kt[:, c0:c0 + 128],
                       qt[:, c0:L], start=(cb == 0), stop=True,
                       skip_group_check=True)
                A1 = []
                for cs, ps1, cb in s_plan:
                    c0 = cs * 128
                    w = L - c0
                    a1 = att.tile([128, L], bf16, tag="a1", name=f"a1_{h}_{cs}")
                    usl = v_u[:, h, cs:cs + 1]
                    if cs in (1, 2):
                        nc.scalar.activation(a1[:, c0:L], ps1[:, cb:cb + w],
                                             Cpy, scale=usl)
                    else:
                        nc.vector.tensor_scalar(a1[:, c0:L], ps1[:, cb:cb + w],
                                                usl, None, MUL)
                    nc.gpsimd.tensor_tensor(a1[:, c0:c0 + 128],
                                            a1[:, c0:c0 + 128], v_mi[:], MUL)
                    A1.append(a1)

                # branch 2 S matrix (independent of branch 1) -------------
                pa2 = psD.tile([128, L], f32, tag="pd", name=f"pa2_{h}")
                pb2 = psD.tile([128, L], f32, tag="pd", name=f"pb2_{h}")
                pc2 = psD.tile([128, L], f32, tag="pd", name=f"pc2_{h}")
                s2_plan = [(0, pa2, 0), (1, pb2, 0), (2, pc2, 0), (3, pc2, 256)]
                for cs, ps2, cb in s2_plan:
                    c0 = cs * 128
                    MM(ps2[:, cb:cb + L - c0], k2t[:, c0:c0 + 128],
                       q2s[:, c0:L], start=(cb == 0), stop=True,
                       skip_group_check=True)
                # packed A2: [cs0 512 | cs1 384 | cs2 256 | cs3 128] * 0.125,
                # strict-causal diag masks at offsets 0/512/896/1152
                a2p = att.tile([128, 1280], bf16, tag="a2", name=f"a2_{h}")
                A2OFF = [0, 512, 896, 1152]
                nc.vector.tensor_scalar(a2p[:, 0:512], pa2[:], 0.125, None, MUL)
                nc.scalar.activation(a2p[:, 512:896], pb2[:, 0:384],
                                     Cpy, scale=0.125)
                nc.scalar.activation(a2p[:, 896:1280], pc2[:, 0:384],
                                     Cpy, scale=0.125)
                for cs in range(NCH):
                    nc.gpsimd.tensor_tensor(a2p[:, A2OFF[cs]:A2OFF[cs] + 128],
                                            a2p[:, A2OFF[cs]:A2OFF[cs] + 128],
                                            v_ms8[:], MUL)

                # O1^T (d-major, gc-scaled via qtg); cols c0:c0+128 of the
                # accumulation are final after the cs-th matmul, so the
                # t-major E extraction pipelines chunk by chunk.  Branch 2
                # then accumulates O2^T INTO THE SAME BANK, so the combined
                # outT needs no add - just one evacuation at the end.
                po = pot.tile([64, L], f32, tag="pot", name=f"po_{h}")
                for cs in range(NCH):
                    c0 = cs * 128
                    MM(po[:, c0:L], v_v[:, h, cs, :], A1[cs][:, c0:L],
                       start=(cs == 0), stop=False,
                       skip_group_check=True)

                Et = att.tile([128, NCH, DH], bf16, tag="et", bufs=2,
                              name=f"et_{h}")
                po1Ts = sm.tile([64, NCH, 128], bf16, tag="po1Ts", bufs=2,
                                name=f"po1Ts_{h}")
                for ct in range(NCH):
                    t0 = ct * 128
                    if ct % 2 == 0:
                        nc.vector.tensor_copy(po1Ts[:, ct, :],
                                              po[:, t0:t0 + 128])
                    else:
                        nc.scalar.copy(po1Ts[:, ct, :], po[:, t0:t0 + 128])
                    ptr = psD.tile([128, DH], bf16, tag="ptr", bufs=1,
                                   name=f"ptr_{h}_{ct}")
                    nc.tensor.transpose(ptr[:], po1Ts[:, ct, :], v_idn)
                    nc.vector.tensor_tensor(Et[:, ct, :], v_vs[:, h, ct, :],
                                            ptr[:], mybir.AluOpType.subtract)

                # O2^T accumulates into po on top of gc*O1^T
                for cs in range(NCH):
                    c0 = cs * 128
                    MM(po[:, c0:L], Et[:, cs, :],
                       a2p[:, A2OFF[cs]:A2OFF[cs] + L - c0],
                       start=False, stop=(cs == NCH - 1),
                       skip_group_check=True)
                # evacuate combined outT split across both engines
                if h == 3:
                    for qd, eng in ((0, 'v'), (1, 'a'), (2, 'v'), (3, 'a')):
                        sl = slice(qd * 128, (qd + 1) * 128)
                        if eng == 'v':
                            nc.vector.tensor_copy(outT2[rows, hp, sl], po[:, sl])
                        else:
                            nc.scalar.copy(outT2[rows, hp, sl], po[:, sl])
                else:
                    nc.vector.tensor_copy(outT2[rows, hp, 0:256], po[:, 0:256])
                    nc.scalar.copy(outT2[rows, hp, 256:L], po[:, 256:L])

            emit_proj(0)
            emit_head(0)
            emit_head(1)
            emit_proj(1)

            # hp=0 halves of the first output-projection tiles can run as
            # soon as heads 0/1 land (psA banks are free after proj 1)
            ysb = qk.tile([128, KC, L], bf16)
            yT_r = yT.ap().rearrange("(c p) t -> p c t", p=128)
            emit_head(2)
            emit_head(3)

            for nci in range(KC):
                n0 = nci * 128
                pool = psA if nci % 2 == 0 else pot
                tg = "pbig" if nci % 2 == 0 else "pot"
                py = pool.tile([128, L], f32, tag=tg, name=f"py_{nci}")
                for hp in range(2):
                    MM(py[:], v_wcp[:, hp, n0:n0 + 128],
                       outT2[:, hp, :], start=(hp == 0), stop=(hp == 1))
                if nci % 2 == 0:
                    nc.vector.tensor_copy(ysb[:, nci, :], py[:])
                else:
                    nc.scalar.copy(ysb[:, nci, :], py[:])
                # single-tile output DMAs, alternating free queues
                eng = nc.sync if nci % 2 == 0 else nc.gpsimd
                eng.dma_start(out=yT_r[:, nci:nci + 1, :],
                              in_=ysb[:, nci:nci + 1, :])

    # this walrus build allows ONE sync wait per engine instruction; Tile's
    # final drain carries the whole vector clock, so split multi-wait
    # instructions into chains of single-wait drains for the HW build.
    if not split_waits:
        return nc
    import concourse.mybir as mybir
    nsplit = 0
    for bb in nc.m.functions[0].blocks:
        new_insts = []
        for inst in bb.instructions:
            si = getattr(inst, "sync_info", None)
            if si is not None and si.on_wait and len(si.on_wait) > 1:
                waits = list(si.on_wait)
                for j, w in enumerate(waits[:-1]):
                    d = mybir.InstDrain(
                        name=f"{inst.name}_sw{j}", engine=inst.engine,
                        ins=[], outs=[],
                        sync_info=mybir.SyncInfo(on_wait=[w], on_update=[]))
                    new_insts.append(d)
                    nsplit += 1
                si.on_wait = [waits[-1]]
            new_insts.append(inst)
        bb.instructions = new_insts
    return nc


# ----------------------------------------------------------------- host side
def _sigmoid(z):
    return 1.0 / (1.0 + np.exp(-z))


def _pack_core(core, x, Gc, U, wq_h, wk_h, wk2_h, wcp_h, bq_h, bk_h, bk2s_h,
               mi, ms8):
    b = core // 4
    h0 = (core % 4) * 4

    pk = np.zeros((128, TOT), dtype=np.float32)

    def put(name, arr):
        a, bb, dt = _REG[name]
        arr = np.asarray(arr)
        arr2 = arr.reshape(arr.shape[0], -1)
        if dt == 'b':
            a16 = arr2.astype(ml_dtypes.bfloat16).view(np.uint16)
            u32 = (a16[:, 0::2].astype(np.uint32)
                   | (a16[:, 1::2].astype(np.uint32) << 16))
            pk[:arr.shape[0], a:a + u32.shape[1]] = u32.view(np.float32)
        elif dt == 'x':
            a8 = arr2.astype(ml_dtypes.float8_e4m3).view(np.uint8)
            u32 = (a8[:, 0::4].astype(np.uint32)
                   | (a8[:, 1::4].astype(np.uint32) << 8)
                   | (a8[:, 2::4].astype(np.uint32) << 16)
                   | (a8[:, 3::4].astype(np.uint32) << 24))
            pk[:arr.shape[0], a:a + u32.shape[1]] = u32.view(np.float32)
        else:
            pk[:arr.shape[0], a:bb] = arr2

    xb = x[b]                                        # (L, D)
    xTc = xb.T.reshape(KC, 128, L).transpose(1, 0, 2)
    put("xT", xTc)
    put("xT8", xTc)
    put("wk28", wk2_h[core])
    put("wq", wq_h[core])
    put("wk", wk_h[core])
    put("wcp", wcp_h[core])

    hsl = slice(h0 * DH, (h0 + 4) * DH)
    vh = xb[:, hsl].reshape(L, 4, DH)
    put("v", vh.reshape(NCH, 128, 4, DH).transpose(1, 2, 0, 3))
    vsh = np.zeros_like(vh)
    vsh[:-1] = vh[1:]
    put("vs", vsh.reshape(NCH, 128, 4, DH).transpose(1, 2, 0, 3))

    gch = Gc[b, :, h0:h0 + 4]                        # (L, 4)
    uh = U[b, :, h0:h0 + 4]
    # gcp[r, hp, t] = Gc[t, hp*2 + r//64]: row-head layout matching QT
    gcp = np.empty((128, 2, L), dtype=np.float32)
    for hp in range(2):
        for l in range(2):
            gcp[l * 64:(l + 1) * 64, hp, :] = gch[:, hp * 2 + l]
    put("gcp", gcp)
    put("u", uh.reshape(NCH, 128, 4).transpose(1, 2, 0))
    put("gc", gch.reshape(NCH, 128, 4).transpose(1, 2, 0))
    put("bq", bq_h[core])
    put("bk", bk_h[core])
    put("bqr", 0.98 * bq_h[core])
    put("bk2s", bk2s_h[core])
    put("mi", mi)
    put("ms8", ms8)
    put("idn", np.eye(64, dtype=np.float32))
    return pk


def kernel(**inputs):
    inputs = {k: np.asarray(v, dtype=np.float32) for k, v in inputs.items()}
    x = inputs["x"]
    q1_w, q1_b = inputs["q1_w"], inputs["q1_b"]
    k1_w, k1_b = inputs["k1_w"], inputs["k1_b"]
    k2_w, k2_b = inputs["k2_w"], inputs["k2_b"]
    gw_w, gw_b = inputs["gw_w"], inputs["gw_b"]
    sw_w, sw_b = inputs["sw_w"], inputs["sw_b"]
    cp_w, cp_b = inputs["cp_w"], inputs["cp_b"]

    # host scalars: gate cumprod Gc and u = silu(K@sw)/Gc  (tiny matvecs)
    xh = x.reshape(B, L, H, DH)
    zg = np.einsum("blhd,d->blh", xh, gw_w[:, 0], optimize=True) + gw_b[0]
    G = _sigmoid(zg)
    log_cp = np.clip(np.cumsum(np.log(np.clip(G, 1e-6, None)), axis=1),
                     -30.0, 30.0)
    Gc = (np.exp(log_cp) + 1e-6).astype(np.float32)        # (B, L, H)

    Wr = (k1_w.reshape(D, H, DH) @ sw_w[:, 0]).astype(np.float32)
    Cr = (k1_b.reshape(H, DH) @ sw_w[:, 0]) + sw_b[0]
    zr = x @ Wr + Cr
    R = zr * _sigmoid(zr)
    U = (R / Gc).astype(np.float32)

    idx = np.arange(128)
    mi = (idx[:, None] <= idx[None, :]).astype(np.float32)
    ms8 = (idx[:, None] < idx[None, :]).astype(np.float32)
    c2 = np.float32(0.02 / np.sqrt(np.float32(D)))

    wq_h, wk_h, wk2_h, wcp_h, bq_h, bk_h, bk2s_h = {}, {}, {}, {}, {}, {}, {}
    for core in range(N_CORES):
        h0 = (core % 4) * 4
        hsl = slice(h0 * DH, (h0 + 4) * DH)
        wq_h[core] = q1_w[:, hsl].reshape(KC, 128, 256).transpose(1, 0, 2)
        wk_h[core] = k1_w[:, hsl].reshape(KC, 128, 256).transpose(1, 0, 2)
        wk2_h[core] = k2_w[:, hsl].reshape(KC, 128, 256).transpose(1, 0, 2)
        wcp_h[core] = cp_w[hsl, :].reshape(2, 128, D).transpose(1, 0, 2)
        bq_h[core] = q1_b[hsl].reshape(2, 128).T
        bk_h[core] = k1_b[hsl].reshape(2, 128).T
        bk2s_h[core] = (k2_b[hsl] * c2).reshape(2, 128).T

    in_maps = [
        {"inp": _pack_core(core, x, Gc, U, wq_h, wk_h, wk2_h, wcp_h,
                           bq_h, bk_h, bk2s_h, mi, ms8)}
        for core in range(N_CORES)
    ]

    _CACHE["last_in_maps"] = in_maps
    res = _run(in_maps)

    y = np.empty((B, L, D), dtype=np.float32)
    for b in range(B):
        acc = res[4 * b]["yT"].astype(np.float32)
        for c in range(4 * b + 1, 4 * b + 4):
            acc = acc + res[c]["yT"]
        y[b] = acc.T
    y += cp_b
    return y


def _run(in_maps, trace=False):
    if "nc" not in _CACHE:
        _CACHE["nc"] = _build_nc()
    from concourse.bass_utils import run_bass_kernel_spmd
    r = run_bass_kernel_spmd(_CACHE["nc"], in_maps,
                             core_ids=list(range(N_CORES)), trace=trace)
    _CACHE["last"] = r
    return r.results


# revision 38
# speedup vs baseline: 1.0067x; 1.0067x over previous
"""GatedLinearAttentionARMA on 8 Trainium2 NeuronCores (Bass/Tile).

B=2, L=512, D=1024, H=16, DH=64.

Both recurrences are linear scans of rank-1 updates, exactly equivalent to
causal quadratic attention:
  O1_t = Gc_t * sum_{s<=t} (Q_t.K_s) * u_s * V_s        (u = silu(K@sw)/Gc)
  E_t  = V_{t+1} - O1_t
  O2_t = sum_{s<t} (q2_{t-1}.k2_s) * E_s
  y    = (O1 + O2) @ cp_w + cp_b

Sharding: core c handles batch b = c//4 and heads (c%4)*4 .. +3.  Each core
emits its heads' d-major partial output yT (1024, 512); the host sums the 4
per-batch partials, transposes, and adds cp_b.

v2 device program (vs the 47.5us baseline):
  - all matmul operands shipped/evacuated as bf16 (PSUM accumulation stays
    f32): halves DMA bytes and makes <256-col matmuls 4x faster on PE.
  - inputs ride 3 parallel DMA queues (SP / Act / Pool) ordered so the first
    projection can start ~3.5us in; outputs are spread over all 3 queues.
  - activation-table warmup at t~0 on a memset tile (the 1.28us table load
    otherwise lands in the first projection evacuation).
  - PSUM evacuations balanced across Act/DVE; masks and m1 on Pool.
"""

import sys

if "/opt/trn_rl_repo" not in sys.path:
    sys.path.insert(0, "/opt/trn_rl_repo")

import numpy as np
import ml_dtypes

B, L, D, H = 2, 512, 1024, 16
DH = D // H            # 64
NCH = L // 128         # 4 time chunks
KC = D // 128          # 8 contraction chunks
N_CORES = 8

# --- packed input column layout (fp32-column units) -----------------------
# dtype 'b' = bf16 (2 elems per fp32 column), 'f' = fp32
_REG = {}
_c = 0
def _alloc(name, cols_f32, dt):
    global _c
    _REG[name] = (_c, _c + cols_f32, dt)
    _c += cols_f32
# Act queue: wq + masks + idn + scalars (needed first by PE / Pool masks)
_alloc("wq", KC * 128, 'b')         # (128, 8, 256) bf16
_alloc("mi", 64, 'b')               # (128, 128) incl causal mask
_alloc("ms8", 64, 'b')              # strict causal mask
_alloc("idn", 32, 'b')              # (64, 64) identity for PE transpose
_alloc("u", 16, 'f')                # (128, 4, 4)
_alloc("gc", 16, 'f')
_alloc("bq", 2, 'f')
_alloc("bk", 2, 'f')
_alloc("bqr", 2, 'f')               # 0.98*bq
_alloc("bk2s", 2, 'f')              # c2*k2_b
R_ACT_END = _c
_alloc("wcp", 2 * D // 2, 'b')      # (128, 2, 1024) bf16  (Act queue tail)
R_ACT2_END = _c
# SP queue: xT (split in 4), xT8 (fp8 copy for the DoubleRow k2 projection)
_alloc("xT", KC * 256, 'b')         # (128, 8, 512) bf16
_alloc("xT8", KC * 128, 'x')        # (128, 4, 2, 512) fp8e4
R_SP_END = _c
# Pool queue: wk, wk28, v, vs, gcp
_alloc("wk", KC * 128, 'b')
_alloc("wk28", KC * 64, 'x')        # (128, 4, 2, 256) fp8e4
_alloc("v", 4 * NCH * DH // 2, 'b')     # (128, 4, 4, 64) bf16
_alloc("vs", 4 * NCH * DH // 2, 'b')
_alloc("gcp", 2 * L // 2, 'b')          # (128p, 2, 512) bf16: Gc per (row-head, hp)
TOT = _c

_CACHE = {}


def _build_nc(split_waits=True):
    import concourse.bass as bass
    import concourse.mybir as mybir
    from concourse.tile import TileContext

    f32 = mybir.dt.float32
    f32r = mybir.dt.float32r
    bf16 = mybir.dt.bfloat16
    nc = bass.Bass()

    inp = nc.dram_tensor("inp", [128, TOT], f32r, kind="ExternalInput")
    yT = nc.dram_tensor("yT", [D, L], bf16, kind="ExternalOutput")

    C2 = 0.02 / float(np.sqrt(D))
    Ident = mybir.ActivationFunctionType.Identity
    Sig = mybir.ActivationFunctionType.Sigmoid
    Rel = mybir.ActivationFunctionType.Relu
    Cpy = mybir.ActivationFunctionType.Copy
    MUL = mybir.AluOpType.mult

    with TileContext(nc) as tc:
        with (
            tc.tile_pool(name="cst", bufs=1) as cst,
            tc.tile_pool(name="qk", bufs=1) as qk,
            tc.tile_pool(name="att", bufs=8) as att,
            tc.tile_pool(name="sm", bufs=4) as sm,
            tc.tile_pool(name="psA", bufs=2, space="PSUM") as psA,
            tc.tile_pool(name="psD", bufs=3, space="PSUM") as psD,
            tc.tile_pool(name="pot", bufs=2, space="PSUM") as pot,
        ):
            mega = cst.tile([128, TOT], f32r)

            # --- input DMAs: 3 parallel queues, first-needed-first --------
            # SP: xT in 4 pieces (drip-feeds the kc-interleaved projections),
            # then wcp.  Act: wq in 2 pieces, then masks+scalars, then free.
            # Pool: wk, wk2, v+vs+gcb.
            a0 = _REG["xT"][0]
            a8 = _REG["xT8"][0]
            for i in range(2):
                nc.sync.dma_start(out=mega[:, a0 + 512 * i:a0 + 512 * (i + 1)],
                                  in_=inp[:, a0 + 512 * i:a0 + 512 * (i + 1)])
            nc.sync.dma_start(out=mega[:, a8:R_SP_END], in_=inp[:, a8:R_SP_END])
            nc.sync.dma_start(out=mega[:, _REG["wcp"][0]:R_ACT2_END],
                              in_=inp[:, _REG["wcp"][0]:R_ACT2_END])
            nc.scalar.dma_start(out=mega[:, 0:128], in_=inp[:, 0:128])
            nc.scalar.dma_start(out=mega[:, 128:1024], in_=inp[:, 128:1024])
            for i in range(2, 4):
                nc.scalar.dma_start(out=mega[:, a0 + 512 * i:a0 + 512 * (i + 1)],
                                    in_=inp[:, a0 + 512 * i:a0 + 512 * (i + 1)])
            nc.scalar.dma_start(out=mega[:, 1024:R_ACT_END],
                                in_=inp[:, 1024:R_ACT_END])
            b0, b1 = _REG["wk"][0], _REG["wk28"][0]
            b2 = _REG["v"][0]
            nc.gpsimd.dma_start(out=mega[:, b0:b1], in_=inp[:, b0:b1])
            nc.gpsimd.dma_start(out=mega[:, b1:b2], in_=inp[:, b1:b2])
            nc.gpsimd.dma_start(out=mega[:, b2:TOT], in_=inp[:, b2:TOT])

            def bview(name, rows, *shape):
                a, b, dt = _REG[name]
                assert dt == 'b'
                ap = mega[0:rows, a:b].bitcast(bf16)
                if len(shape) > 1:
                    ap = ap.rearrange(
                        "p (" + " ".join(f"d{i}" for i in range(len(shape))) + ") -> p "
                        + " ".join(f"d{i}" for i in range(len(shape))),
                        **{f"d{i}": s for i, s in enumerate(shape)},
                    )
                return ap

            fp8 = mybir.dt.float8e4

            def xview(name, *shape):
                a, b, dt_ = _REG[name]
                ap = mega[:, a:b].bitcast(fp8)
                return ap.rearrange(
                    "p (" + " ".join(f"d{i}" for i in range(len(shape))) + ") -> p "
                    + " ".join(f"d{i}" for i in range(len(shape))),
                    **{f"d{i}": s for i, s in enumerate(shape)},
                )

            v_xT = bview("xT", 128, KC, L)
            v_xT8 = xview("xT8", KC // 2, 2, L)
            v_wk28 = xview("wk28", KC // 2, 2, 256)
            v_wq = bview("wq", 128, KC, 256)
            v_wk = bview("wk", 128, KC, 256)
            v_wcp = bview("wcp", 128, 2, D)
            v_v = bview("v", 128, 4, NCH, DH)
            v_vs = bview("vs", 128, 4, NCH, DH)
            v_gcp = bview("gcp", 128, 2, L)
            v_mi = bview("mi", 128, 128)
            v_ms8 = bview("ms8", 128, 128)
            v_idn = bview("idn", 64, 64)

            # --- act-table warmup at t~0 (Sigmoid loads the shared table) -
            wrm = sm.tile([1, 2], f32, tag="wrm", bufs=1)
            nc.gpsimd.memset(wrm[:], 0.25)
            nc.scalar.activation(wrm[0:1, 1:2], wrm[0:1, 0:1], Sig, scale=1.0)

            # --- PE p-state warmup: ~2us of dummy matmuls on a memset tile
            # (PE ramps 0.65->1.2->2.4GHz over 3us of continuous activity;
            # burn the ramp before the first weights arrive).
            grb = cst.tile([128, 256], bf16)
            nc.vector.memset(grb[:], 0.125)
            pdum = psD.tile([128, 256], f32, tag="ptr", bufs=1, name="pdum")
            NDUM = 9
            for i in range(NDUM):
                nc.tensor.matmul(pdum[:], grb[:, 0:128], grb[:, 0:256],
                                 start=(i == 0), stop=(i == NDUM - 1))

            # f32 copies of the per-partition scalar block (tensor_scalar and
            # activation bias operands must be plain float32)
            sc = cst.tile([128, 32], f32)    # u(16) + gc(16)
            nc.vector.tensor_copy(sc[:], mega[:, _REG["u"][0]:_REG["gc"][1]])
            scA = cst.tile([128, 8], f32)    # bq bk bqr bk2s
            nc.scalar.copy(scA[:], mega[:, _REG["bq"][0]:_REG["bk2s"][1]])
            v_u = sc[:, 0:16].rearrange("p (h c) -> p h c", h=4)
            v_gc = sc[:, 16:32].rearrange("p (h c) -> p h c", h=4)
            v_bq = scA[:, 0:2]
            v_bk = scA[:, 2:4]
            v_bqr = scA[:, 4:6]
            v_bk2s = scA[:, 6:8]

            def MM(out, lhsT, rhs, **kw):
                return nc.tensor.matmul(out, lhsT, rhs, **kw)

            # ---- projections: d-major QT/KT/k2T (2 heads per tile) ------
            QT = qk.tile([128, 2, L], bf16)
            QTG = qk.tile([128, 2, L], bf16)   # QT * Gc_t (per row-head)
            KT = qk.tile([128, 2, L], bf16)
            K2T = qk.tile([128, 2, L], bf16)
            Q2M = qk.tile([128, 2, L], bf16)   # min(Q, 0.02Q)
            Q2S = qk.tile([128, 2, L], bf16)   # Q2M shifted right by one

            def emit_proj(hp):
                # q and k matmuls interleaved kc-major so the xT chunks are
                # consumed as they arrive off the drip-fed SP queue.
                mcol = slice(hp * 128, (hp + 1) * 128)
                pq = psA.tile([128, L], f32, tag="pbig", name=f"pq_{hp}")
                pk = psA.tile([128, L], f32, tag="pbig", name=f"pk_{hp}")
                for kc in range(KC):
                    MM(pq[:], v_wq[:, kc, mcol], v_xT[:, kc, :],
                       start=(kc == 0), stop=(kc == KC - 1))
                nc.scalar.activation(QT[:, hp, :], pq[:], Ident,
                                     bias=v_bq[:, hp:hp + 1], scale=1.0)
                for kc in range(KC):
                    MM(pk[:], v_wk[:, kc, mcol], v_xT[:, kc, :],
                       start=(kc == 0), stop=(kc == KC - 1))
                nc.scalar.activation(KT[:, hp, :], pk[:], Ident,
                                     bias=v_bk[:, hp:hp + 1], scale=1.0)
                # branch-1 Q carries the gate: qtg_t = Q_t * Gc_t
                nc.vector.tensor_tensor(QTG[:, hp, :], QT[:, hp, :],
                                        v_gcp[:, hp, :], MUL)

                pk2 = psA.tile([128, L], f32, tag="pbig", name=f"pk2_{hp}")
                for kc2 in range(KC // 2):
                    MM(pk2[:], v_wk28[:, kc2, :, mcol], v_xT8[:, kc2, :, :],
                       start=(kc2 == 0), stop=(kc2 == KC // 2 - 1),
                       perf_mode=mybir.MatmulPerfMode.DoubleRow)
                nc.scalar.activation(K2T[:, hp, :], pk2[:], Sig,
                                     bias=v_bk2s[:, hp:hp + 1], scale=C2)

                # q2m = min(Q, 0.02*Q)   (true q2 * 8)
                Q02 = sm.tile([128, L], bf16, tag="q02", bufs=2,
                              name=f"q02_{hp}")
                nc.vector.tensor_scalar(Q02[:], QT[:, hp, :], 0.02, None, MUL)
                nc.vector.tensor_tensor(Q2M[:, hp, :], QT[:, hp, :], Q02[:],
                                        mybir.AluOpType.min)
                nc.vector.tensor_copy(Q2S[:, hp, 1:L], Q2M[:, hp, 0:L - 1])
                nc.vector.tensor_scalar(Q2S[:, hp, 0:1], QT[:, hp, 0:1],
                                        0.0, None, MUL)

            # ---- per-head attention ------------------------------------
            outT2 = att.tile([128, 2, L], bf16, tag="otp", bufs=1)

            def emit_head(h):
                hp, r0 = h // 2, (h % 2) * 64
                rows = slice(r0, r0 + 64)
                qt = QTG[rows, hp, :]
                kt = KT[rows, hp, :]
                k2t = K2T[rows, hp, :]
                q2s = Q2S[rows, hp, :]

                # branch 1: S^T chunks -> A1 (u-scaled, causal incl).
                # cs=2 and cs=3 share one PSUM bank (column-packed).
                pa = psD.tile([128, L], f32, tag="pd", name=f"pa1_{h}")
                pb = psD.tile([128, L], f32, tag="pd", name=f"pb1_{h}")
                pc = psD.tile([128, L], f32, tag="pd", name=f"pc1_{h}")
                s_plan = [(0, pa, 0), (1, pb, 0), (2, pc, 0), (3, pc, 256)]
                for cs, ps1, cb in s_plan:
                    c0 = cs * 128
                    MM(ps1[:, cb:cb + L - c0], kt[:, c0:c0 + 128],
                       qt[:, c0:L], start=(cb == 0), stop=True,
                       skip_group_check=True)
                A1 = []
                for cs, ps1, cb in s_plan:
                    c0 = cs * 128
                    w = L - c0
                    a1 = att.tile([128, L], bf16, tag="a1", name=f"a1_{h}_{cs}")
                    usl = v_u[:, h, cs:cs + 1]
                    if cs in (1, 2):
                        nc.scalar.activation(a1[:, c0:L], ps1[:, cb:cb + w],
                                             Cpy, scale=usl)
                    else:
                        nc.vector.tensor_scalar(a1[:, c0:L], ps1[:, cb:cb + w],
                                                usl, None, MUL)
                    nc.gpsimd.tensor_tensor(a1[:, c0:c0 + 128],
                                            a1[:, c0:c0 + 128], v_mi[:], MUL)
                    A1.append(a1)

                # branch 2 S matrix (independent of branch 1) -------------
                pa2 = psD.tile([128, L], f32, tag="pd", name=f"pa2_{h}")
                pb2 = psD.tile([128, L], f32, tag="pd", name=f"pb2_{h}")
                pc2 = psD.tile([128, L], f32, tag="pd", name=f"pc2_{h}")
                s2_plan = [(0, pa2, 0), (1, pb2, 0), (2, pc2, 0), (3, pc2, 256)]
                for cs, ps2, cb in s2_plan:
                    c0 = cs * 128
                    MM(ps2[:, cb:cb + L - c0], k2t[:, c0:c0 + 128],
                       q2s[:, c0:L], start=(cb == 0), stop=True,
                       skip_group_check=True)
                # packed A2: [cs0 512 | cs1 384 | cs2 256 | cs3 128] * 0.125,
                # strict-causal diag masks at offsets 0/512/896/1152
                a2p = att.tile([128, 1280], bf16, tag="a2", name=f"a2_{h}")
                A2OFF = [0, 512, 896, 1152]
                nc.vector.tensor_scalar(a2p[:, 0:512], pa2[:], 0.125, None, MUL)
                nc.scalar.activation(a2p[:, 512:896], pb2[:, 0:384],
                                     Cpy, scale=0.125)
                nc.scalar.activation(a2p[:, 896:1280], pc2[:, 0:384],
                                     Cpy, scale=0.125)
                for cs in range(NCH):
                    nc.gpsimd.tensor_tensor(a2p[:, A2OFF[cs]:A2OFF[cs] + 128],
                                            a2p[:, A2OFF[cs]:A2OFF[cs] + 128],
                                            v_ms8[:], MUL)

                # O1^T (d-major, gc-scaled via qtg); cols c0:c0+128 of the
                # accumulation are final after the cs-th matmul, so the
                # t-major E extraction pipelines chunk by chunk.  Branch 2
                # then accumulates O2^T INTO THE SAME BANK, so the combined
                # outT needs no add - just one evacuation at the end.
                po = pot.tile([64, L], f32, tag="pot", name=f"po_{h}")
                for cs in range(NCH):
                    c0 = cs * 128
                    MM(po[:, c0:L], v_v[:, h, cs, :], A1[cs][:, c0:L],
                       start=(cs == 0), stop=False,
                       skip_group_check=True)

                Et = att.tile([128, NCH, DH], bf16, tag="et", bufs=2,
                              name=f"et_{h}")
                po1Ts = sm.tile([64, NCH, 128], bf16, tag="po1Ts", bufs=2,
                                name=f"po1Ts_{h}")
                for ct in range(NCH):
                    t0 = ct * 128
                    if ct % 2 == 0:
                        nc.vector.tensor_copy(po1Ts[:, ct, :],
                                              po[:, t0:t0 + 128])
                    else:
                        nc.scalar.copy(po1Ts[:, ct, :], po[:, t0:t0 + 128])
                    ptr = psD.tile([128, DH], bf16, tag="ptr", bufs=1,
                                   name=f"ptr_{h}_{ct}")
                    nc.tensor.transpose(ptr[:], po1Ts[:, ct, :], v_idn)
                    nc.vector.tensor_tensor(Et[:, ct, :], v_vs[:, h, ct, :],
                                            ptr[:], mybir.AluOpType.subtract)

                # O2^T accumulates into po on top of gc*O1^T
                for cs in range(NCH):
                    c0 = cs * 128
                    MM(po[:, c0:L], Et[:, cs, :],
                       a2p[:, A2OFF[cs]:A2OFF[cs] + L - c0],
                       start=False, stop=(cs == NCH - 1),
                       skip_group_check=True)
                # evacuate combined outT split across both engines
                if h == 3:
                    for qd, eng in ((0, 'v'), (1, 'a'), (2, 'v'), (3, 'a')):
                        sl = slice(qd * 128, (qd + 1) * 128)
                        if eng == 'v':
                            nc.vector.tensor_copy(outT2[rows, hp, sl], po[:, sl])
                        else:
                            nc.scalar.copy(outT2[rows, hp, sl], po[:, sl])
                else:
                    nc.vector.tensor_copy(outT2[rows, hp, 0:256], po[:, 0:256])
                    nc.scalar.copy(outT2[rows, hp, 256:L], po[:, 256:L])

            emit_proj(0)
            emit_head(0)
            emit_head(1)
            emit_proj(1)

            # hp=0 halves of the first output-projection tiles can run as
            # soon as heads 0/1 land (psA banks are free after proj 1)
            ysb = qk.tile([128, KC, L], bf16)
            yT_r = yT.ap().rearrange("(c p) t -> p c t", p=128)
            emit_head(2)
            emit_head(3)

            for nci in range(KC):
                n0 = nci * 128
                pool = psA if nci % 2 == 0 else pot
                tg = "pbig" if nci % 2 == 0 else "pot"
                py = pool.tile([128, L], f32, tag=tg, name=f"py_{nci}")
                for hp in range(2):
                    MM(py[:], v_wcp[:, hp, n0:n0 + 128],
                       outT2[:, hp, :], start=(hp == 0), stop=(hp == 1))
                if nci % 2 == 0:
                    nc.vector.tensor_copy(ysb[:, nci, :], py[:])
                else:
                    nc.scalar.copy(ysb[:, nci, :], py[:])
                # single-tile output DMAs, alternating free queues
                eng = nc.sync if nci % 2 == 0 else nc.gpsimd
                eng.dma_start(out=yT_r[:, nci:nci + 1, :],
                              in_=ysb[:, nci:nci + 1, :])

    # this walrus build allows ONE sync wait per engine instruction; Tile's
    # final drain carries the whole vector clock, so split multi-wait
    # instructions into chains of single-wait drains for the HW build.
    if not split_waits:
        return nc
    import concourse.mybir as mybir
    nsplit = 0
    for bb in nc.m.functions[0].blocks:
        new_insts = []
        for inst in bb.instructions:
            si = getattr(inst, "sync_info", None)
            if si is not None and si.on_wait and len(si.on_wait) > 1:
                waits = list(si.on_wait)
                for j, w in enumerate(waits[:-1]):
                    d = mybir.InstDrain(
                        name=f"{inst.name}_sw{j}", engine=inst.engine,
                        ins=[], outs=[],
                        sync_info=mybir.SyncInfo(on_wait=[w], on_update=[]))
                    new_insts.append(d)
                    nsplit += 1
                si.on_wait = [waits[-1]]
            new_insts.append(inst)
        bb.instructions = new_insts
    return nc


# ----------------------------------------------------------------- host side
def _sigmoid(z):
    return 1.0 / (1.0 + np.exp(-z))


def _pack_core(core, x, Gc, U, wq_h, wk_h, wk2_h, wcp_h, bq_h, bk_h, bk2s_h,
               mi, ms8):
    b = core // 4
    h0 = (core % 4) * 4

    pk = np.zeros((128, TOT), dtype=np.float32)

    def put(name, arr):
        a, bb, dt = _REG[name]
        arr = np.asarray(arr)
        arr2 = arr.reshape(arr.shape[0], -1)
        if dt == 'b':
            a16 = arr2.astype(ml_dtypes.bfloat16).view(np.uint16)
            u32 = (a16[:, 0::2].astype(np.uint32)
                   | (a16[:, 1::2].astype(np.uint32) << 16))
            pk[:arr.shape[0], a:a + u32.shape[1]] = u32.view(np.float32)
        elif dt == 'x':
            a8 = arr2.astype(ml_dtypes.float8_e4m3).view(np.uint8)
            u32 = (a8[:, 0::4].astype(np.uint32)
                   | (a8[:, 1::4].astype(np.uint32) << 8)
                   | (a8[:, 2::4].astype(np.uint32) << 16)
                   | (a8[:, 3::4].astype(np.uint32) << 24))
            pk[:arr.shape[0], a:a + u32.shape[1]] = u32.view(np.float32)
        else:
            pk[:arr.shape[0], a:bb] = arr2

    xb = x[b]                                        # (L, D)
    xTc = xb.T.reshape(KC, 128, L).transpose(1, 0, 2)
    put("xT", xTc)
    put("xT8", xTc)
    put("wk28", wk2_h[core])
    put("wq", wq_h[core])
    put("wk", wk_h[core])
    put("wcp", wcp_h[core])

    hsl = slice(h0 * DH, (h0 + 4) * DH)
    vh = xb[:, hsl].reshape(L, 4, DH)
    put("v", vh.reshape(NCH, 128, 4, DH).transpose(1, 2, 0, 3))
    vsh = np.zeros_like(vh)
    vsh[:-1] = vh[1:]
    put("vs", vsh.reshape(NCH, 128, 4, DH).transpose(1, 2, 0, 3))

    gch = Gc[b, :, h0:h0 + 4]                        # (L, 4)
    uh = U[b, :, h0:h0 + 4]
    # gcp[r, hp, t] = Gc[t, hp*2 + r//64]: row-head layout matching QT
    gcp = np.empty((128, 2, L), dtype=np.float32)
    for hp in range(2):
        for l in range(2):
            gcp[l * 64:(l + 1) * 64, hp, :] = gch[:, hp * 2 + l]
    put("gcp", gcp)
    put("u", uh.reshape(NCH, 128, 4).transpose(1, 2, 0))
    put("gc", gch.reshape(NCH, 128, 4).transpose(1, 2, 0))
    put("bq", bq_h[core])
    put("bk", bk_h[core])
    put("bqr", 0.98 * bq_h[core])
    put("bk2s", bk2s_h[core])
    put("mi", mi)
    put("ms8", ms8)
    put("idn", np.eye(64, dtype=np.float32))
    return pk


def kernel(**inputs):
    inputs = {k: np.asarray(v, dtype=np.float32) for k, v in inputs.items()}
    x = inputs["x"]
    q1_w, q1_b = inputs["q1_w"], inputs["q1_b"]
    k1_w, k1_b = inputs["k1_w"], inputs["k1_b"]
    k2_w, k2_b = inputs["k2_w"], inputs["k2_b"]
    gw_w, gw_b = inputs["gw_w"], inputs["gw_b"]
    sw_w, sw_b = inputs["sw_w"], inputs["sw_b"]
    cp_w, cp_b = inputs["cp_w"], inputs["cp_b"]

    # host scalars: gate cumprod Gc and u = silu(K@sw)/Gc  (tiny matvecs)
    xh = x.reshape(B, L, H, DH)
    zg = np.einsum("blhd,d->blh", xh, gw_w[:, 0], optimize=True) + gw_b[0]
    G = _sigmoid(zg)
    log_cp = np.clip(np.cumsum(np.log(np.clip(G, 1e-6, None)), axis=1),
                     -30.0, 30.0)
    Gc = (np.exp(log_cp) + 1e-6).astype(np.float32)        # (B, L, H)

    Wr = (k1_w.reshape(D, H, DH) @ sw_w[:, 0]).astype(np.float32)
    Cr = (k1_b.reshape(H, DH) @ sw_w[:, 0]) + sw_b[0]
    zr = x @ Wr + Cr
    R = zr * _sigmoid(zr)
    U = (R / Gc).astype(np.float32)

    idx = np.arange(128)
    mi = (idx[:, None] <= idx[None, :]).astype(np.float32)
    ms8 = (idx[:, None] < idx[None, :]).astype(np.float32)
    c2 = np.float32(0.02 / np.sqrt(np.float32(D)))

    wq_h, wk_h, wk2_h, wcp_h, bq_h, bk_h, bk2s_h = {}, {}, {}, {}, {}, {}, {}
    for core in range(N_CORES):
        h0 = (core % 4) * 4
        hsl = slice(h0 * DH, (h0 + 4) * DH)
        wq_h[core] = q1_w[:, hsl].reshape(KC, 128, 256).transpose(1, 0, 2)
        wk_h[core] = k1_w[:, hsl].reshape(KC, 128, 256).transpose(1, 0, 2)
        wk2_h[core] = k2_w[:, hsl].reshape(KC, 128, 256).transpose(1, 0, 2)
        wcp_h[core] = cp_w[hsl, :].reshape(2, 128, D).transpose(1, 0, 2)
        bq_h[core] = q1_b[hsl].reshape(2, 128).T
        bk_h[core] = k1_b[hsl].reshape(2, 128).T
        bk2s_h[core] = (k2_b[hsl] * c2).reshape(2, 128).T

    in_maps = [
        {"inp": _pack_core(core, x, Gc, U, wq_h, wk_h, wk2_h, wcp_h,
                           bq_h, bk_h, bk2s_h, mi, ms8)}
        for core in range(N_CORES)
    ]

    _CACHE["last_in_maps"] = in_maps
    res = _run(in_maps)

    y = np.empty((B, L, D), dtype=np.float32)
    for b in range(B):
        acc = res[4 * b]["yT"].astype(np.float32)
        for c in range(4 * b + 1, 4 * b + 4):
            acc = acc + res[c]["yT"]
        y[b] = acc.T
    y += cp_b
    return y


def _run(in_maps, trace=False):
    if "nc" not in _CACHE:
        _CACHE["nc"] = _build_nc()
    from concourse.bass_utils import run_bass_kernel_spmd
    r = run_bass_kernel_spmd(_CACHE["nc"], in_maps,
                             core_ids=list(range(N_CORES)), trace=trace)
    _CACHE["last"] = r
    return r.results


# revision 39
# speedup vs baseline: 1.0226x; 1.0158x over previous
"""GatedLinearAttentionARMA on 8 Trainium2 NeuronCores (Bass/Tile).

B=2, L=512, D=1024, H=16, DH=64.

Both recurrences are linear scans of rank-1 updates, exactly equivalent to
causal quadratic attention:
  O1_t = Gc_t * sum_{s<=t} (Q_t.K_s) * u_s * V_s        (u = silu(K@sw)/Gc)
  E_t  = V_{t+1} - O1_t
  O2_t = sum_{s<t} (q2_{t-1}.k2_s) * E_s
  y    = (O1 + O2) @ cp_w + cp_b

Sharding: core c handles batch b = c//4 and heads (c%4)*4 .. +3.  Each core
emits its heads' d-major partial output yT (1024, 512); the host sums the 4
per-batch partials, transposes, and adds cp_b.

v2 device program (vs the 47.5us baseline):
  - all matmul operands shipped/evacuated as bf16 (PSUM accumulation stays
    f32): halves DMA bytes and makes <256-col matmuls 4x faster on PE.
  - inputs ride 3 parallel DMA queues (SP / Act / Pool) ordered so the first
    projection can start ~3.5us in; outputs are spread over all 3 queues.
  - activation-table warmup at t~0 on a memset tile (the 1.28us table load
    otherwise lands in the first projection evacuation).
  - PSUM evacuations balanced across Act/DVE; masks and m1 on Pool.
"""

import sys

if "/opt/trn_rl_repo" not in sys.path:
    sys.path.insert(0, "/opt/trn_rl_repo")

import numpy as np
import ml_dtypes

B, L, D, H = 2, 512, 1024, 16
DH = D // H            # 64
NCH = L // 128         # 4 time chunks
KC = D // 128          # 8 contraction chunks
N_CORES = 8

# --- packed input column layout (fp32-column units) -----------------------
# dtype 'b' = bf16 (2 elems per fp32 column), 'f' = fp32
_REG = {}
_c = 0
def _alloc(name, cols_f32, dt):
    global _c
    _REG[name] = (_c, _c + cols_f32, dt)
    _c += cols_f32
# Act queue: wq + masks + idn + scalars (needed first by PE / Pool masks)
_alloc("wq", KC * 128, 'b')         # (128, 8, 256) bf16
_alloc("mi", 64, 'b')               # (128, 128) incl causal mask
_alloc("ms8", 64, 'b')              # strict causal mask
_alloc("idn", 32, 'b')              # (64, 64) identity for PE transpose
_alloc("u", 16, 'f')                # (128, 4, 4)
_alloc("gc", 16, 'f')
_alloc("bq", 2, 'f')
_alloc("bk", 2, 'f')
_alloc("bqr", 2, 'f')               # 0.98*bq
_alloc("bk2s", 2, 'f')              # c2*k2_b
R_ACT_END = _c
_alloc("wcp", 2 * D // 2, 'b')      # (128, 2, 1024) bf16  (Act queue tail)
R_ACT2_END = _c
# SP queue: xT (split in 4), xT8 (fp8 copy for the DoubleRow k2 projection)
_alloc("xT", KC * 256, 'b')         # (128, 8, 512) bf16
_alloc("xT8", KC * 128, 'x')        # (128, 4, 2, 512) fp8e4
R_SP_END = _c
# Pool queue: wk, wk28, v, vs, gcp
_alloc("wk", KC * 128, 'b')
_alloc("wk28", KC * 64, 'x')        # (128, 4, 2, 256) fp8e4
_alloc("v", 4 * NCH * DH // 2, 'b')     # (128, 4, 4, 64) bf16
_alloc("vs", 4 * NCH * DH // 2, 'b')
_alloc("gcp", 2 * L // 2, 'b')          # (128p, 2, 512) bf16: Gc per (row-head, hp)
TOT = _c

_CACHE = {}


def _build_nc(split_waits=True):
    import concourse.bass as bass
    import concourse.mybir as mybir
    from concourse.tile import TileContext

    f32 = mybir.dt.float32
    f32r = mybir.dt.float32r
    bf16 = mybir.dt.bfloat16
    nc = bass.Bass()

    inp = nc.dram_tensor("inp", [128, TOT], f32r, kind="ExternalInput")
    yT = nc.dram_tensor("yT", [D, L], bf16, kind="ExternalOutput")

    C2 = 0.02 / float(np.sqrt(D))
    Ident = mybir.ActivationFunctionType.Identity
    Sig = mybir.ActivationFunctionType.Sigmoid
    Rel = mybir.ActivationFunctionType.Relu
    Cpy = mybir.ActivationFunctionType.Copy
    MUL = mybir.AluOpType.mult

    with TileContext(nc) as tc:
        with (
            tc.tile_pool(name="cst", bufs=1) as cst,
            tc.tile_pool(name="qk", bufs=1) as qk,
            tc.tile_pool(name="att", bufs=8) as att,
            tc.tile_pool(name="sm", bufs=4) as sm,
            tc.tile_pool(name="psA", bufs=2, space="PSUM") as psA,
            tc.tile_pool(name="psD", bufs=3, space="PSUM") as psD,
            tc.tile_pool(name="pot", bufs=2, space="PSUM") as pot,
        ):
            mega = cst.tile([128, TOT], f32r)

            # --- input DMAs: 3 parallel queues, first-needed-first --------
            # SP: xT in 4 pieces (drip-feeds the kc-interleaved projections),
            # then wcp.  Act: wq in 2 pieces, then masks+scalars, then free.
            # Pool: wk, wk2, v+vs+gcb.
            a0 = _REG["xT"][0]
            a8 = _REG["xT8"][0]
            for i in range(2):
                nc.sync.dma_start(out=mega[:, a0 + 512 * i:a0 + 512 * (i + 1)],
                                  in_=inp[:, a0 + 512 * i:a0 + 512 * (i + 1)])
            nc.sync.dma_start(out=mega[:, a8:R_SP_END], in_=inp[:, a8:R_SP_END])
            nc.sync.dma_start(out=mega[:, _REG["wcp"][0]:R_ACT2_END],
                              in_=inp[:, _REG["wcp"][0]:R_ACT2_END])
            nc.scalar.dma_start(out=mega[:, 0:128], in_=inp[:, 0:128])
            nc.scalar.dma_start(out=mega[:, 128:1024], in_=inp[:, 128:1024])
            for i in range(2, 4):
                nc.scalar.dma_start(out=mega[:, a0 + 512 * i:a0 + 512 * (i + 1)],
                                    in_=inp[:, a0 + 512 * i:a0 + 512 * (i + 1)])
            nc.scalar.dma_start(out=mega[:, 1024:R_ACT_END],
                                in_=inp[:, 1024:R_ACT_END])
            b0, b1 = _REG["wk"][0], _REG["wk28"][0]
            b2 = _REG["v"][0]
            nc.gpsimd.dma_start(out=mega[:, b0:b1], in_=inp[:, b0:b1])
            nc.gpsimd.dma_start(out=mega[:, b1:b2], in_=inp[:, b1:b2])
            nc.gpsimd.dma_start(out=mega[:, b2:TOT], in_=inp[:, b2:TOT])

            def bview(name, rows, *shape):
                a, b, dt = _REG[name]
                assert dt == 'b'
                ap = mega[0:rows, a:b].bitcast(bf16)
                if len(shape) > 1:
                    ap = ap.rearrange(
                        "p (" + " ".join(f"d{i}" for i in range(len(shape))) + ") -> p "
                        + " ".join(f"d{i}" for i in range(len(shape))),
                        **{f"d{i}": s for i, s in enumerate(shape)},
                    )
                return ap

            fp8 = mybir.dt.float8e4

            def xview(name, *shape):
                a, b, dt_ = _REG[name]
                ap = mega[:, a:b].bitcast(fp8)
                return ap.rearrange(
                    "p (" + " ".join(f"d{i}" for i in range(len(shape))) + ") -> p "
                    + " ".join(f"d{i}" for i in range(len(shape))),
                    **{f"d{i}": s for i, s in enumerate(shape)},
                )

            v_xT = bview("xT", 128, KC, L)
            v_xT8 = xview("xT8", KC // 2, 2, L)
            v_wk28 = xview("wk28", KC // 2, 2, 256)
            v_wq = bview("wq", 128, KC, 256)
            v_wk = bview("wk", 128, KC, 256)
            v_wcp = bview("wcp", 128, 2, D)
            v_v = bview("v", 128, 4, NCH, DH)
            v_vs = bview("vs", 128, 4, NCH, DH)
            v_gcp = bview("gcp", 128, 2, L)
            v_mi = bview("mi", 128, 128)
            v_ms8 = bview("ms8", 128, 128)
            v_idn = bview("idn", 64, 64)

            # --- act-table warmup at t~0 (Sigmoid loads the shared table) -
            wrm = sm.tile([1, 2], f32, tag="wrm", bufs=1)
            nc.gpsimd.memset(wrm[:], 0.25)
            nc.scalar.activation(wrm[0:1, 1:2], wrm[0:1, 0:1], Sig, scale=1.0)

            # --- PE p-state warmup: ~2us of dummy matmuls on a memset tile
            # (PE ramps 0.65->1.2->2.4GHz over 3us of continuous activity;
            # burn the ramp before the first weights arrive).
            grb = cst.tile([128, 256], bf16)
            nc.vector.memset(grb[:], 0.125)
            pdum = psD.tile([128, 256], f32, tag="ptr", bufs=1, name="pdum")
            NDUM = 9
            for i in range(NDUM):
                nc.tensor.matmul(pdum[:], grb[:, 0:128], grb[:, 0:256],
                                 start=(i == 0), stop=(i == NDUM - 1))

            # f32 copies of the per-partition scalar block (tensor_scalar and
            # activation bias operands must be plain float32)
            sc = cst.tile([128, 32], f32)    # u(16) + gc(16)
            nc.vector.tensor_copy(sc[:], mega[:, _REG["u"][0]:_REG["gc"][1]])
            scA = cst.tile([128, 8], f32)    # bq bk bqr bk2s
            nc.scalar.copy(scA[:], mega[:, _REG["bq"][0]:_REG["bk2s"][1]])
            v_u = sc[:, 0:16].rearrange("p (h c) -> p h c", h=4)
            v_gc = sc[:, 16:32].rearrange("p (h c) -> p h c", h=4)
            v_bq = scA[:, 0:2]
            v_bk = scA[:, 2:4]
            v_bqr = scA[:, 4:6]
            v_bk2s = scA[:, 6:8]

            def MM(out, lhsT, rhs, **kw):
                return nc.tensor.matmul(out, lhsT, rhs, **kw)

            # ---- projections: d-major QT/KT/k2T (2 heads per tile) ------
            QT = qk.tile([128, 2, L], bf16)
            QTG = qk.tile([128, 2, L], bf16)   # QT * Gc_t (per row-head)
            KT = qk.tile([128, 2, L], bf16)
            K2T = qk.tile([128, 2, L], bf16)
            Q2M = qk.tile([128, 2, L], bf16)   # min(Q, 0.02Q)
            Q2S = qk.tile([128, 2, L], bf16)   # Q2M shifted right by one

            def emit_proj(hp):
                # q and k matmuls interleaved kc-major so the xT chunks are
                # consumed as they arrive off the drip-fed SP queue.
                mcol = slice(hp * 128, (hp + 1) * 128)
                pq = psA.tile([128, L], f32, tag="pbig", name=f"pq_{hp}")
                pk = psA.tile([128, L], f32, tag="pbig", name=f"pk_{hp}")
                for kc in range(KC):
                    MM(pq[:], v_wq[:, kc, mcol], v_xT[:, kc, :],
                       start=(kc == 0), stop=(kc == KC - 1))
                nc.scalar.activation(QT[:, hp, :], pq[:], Ident,
                                     bias=v_bq[:, hp:hp + 1], scale=1.0)
                for kc in range(KC):
                    MM(pk[:], v_wk[:, kc, mcol], v_xT[:, kc, :],
                       start=(kc == 0), stop=(kc == KC - 1))
                nc.scalar.activation(KT[:, hp, :], pk[:], Ident,
                                     bias=v_bk[:, hp:hp + 1], scale=1.0)
                # branch-1 Q carries the gate: qtg_t = Q_t * Gc_t
                nc.vector.tensor_tensor(QTG[:, hp, :], QT[:, hp, :],
                                        v_gcp[:, hp, :], MUL)

                pk2 = psA.tile([128, L], f32, tag="pbig", name=f"pk2_{hp}")
                for kc2 in range(KC // 2):
                    MM(pk2[:], v_wk28[:, kc2, :, mcol], v_xT8[:, kc2, :, :],
                       start=(kc2 == 0), stop=(kc2 == KC // 2 - 1),
                       perf_mode=mybir.MatmulPerfMode.DoubleRow)
                nc.scalar.activation(K2T[:, hp, :], pk2[:], Sig,
                                     bias=v_bk2s[:, hp:hp + 1], scale=C2)

                # q2m = min(Q, 0.02*Q)   (true q2 * 8)
                Q02 = sm.tile([128, L], bf16, tag="q02", bufs=2,
                              name=f"q02_{hp}")
                nc.vector.tensor_scalar(Q02[:], QT[:, hp, :], 0.02, None, MUL)
                nc.vector.tensor_tensor(Q2M[:, hp, :], QT[:, hp, :], Q02[:],
                                        mybir.AluOpType.min)
                nc.vector.tensor_copy(Q2S[:, hp, 1:L], Q2M[:, hp, 0:L - 1])
                nc.vector.tensor_scalar(Q2S[:, hp, 0:1], QT[:, hp, 0:1],
                                        0.0, None, MUL)

            # ---- per-head attention ------------------------------------
            outT2 = att.tile([128, 2, L], bf16, tag="otp", bufs=1)

            def emit_head(h):
                hp, r0 = h // 2, (h % 2) * 64
                rows = slice(r0, r0 + 64)
                qt = QTG[rows, hp, :]
                kt = KT[rows, hp, :]
                k2t = K2T[rows, hp, :]
                q2s = Q2S[rows, hp, :]

                # branch 1: S^T chunks -> A1 (u-scaled, causal incl).
                # cs=2 and cs=3 share one PSUM bank (column-packed).
                pa = psD.tile([128, L], f32, tag="pd", name=f"pa1_{h}")
                pb = psD.tile([128, L], f32, tag="pd", name=f"pb1_{h}")
                pc = psD.tile([128, L], f32, tag="pd", name=f"pc1_{h}")
                s_plan = [(0, pa, 0), (1, pb, 0), (2, pc, 0), (3, pc, 256)]
                for cs, ps1, cb in s_plan:
                    c0 = cs * 128
                    MM(ps1[:, cb:cb + L - c0], kt[:, c0:c0 + 128],
                       qt[:, c0:L], start=(cb == 0), stop=True,
                       skip_group_check=True)
                A1 = []
                for cs, ps1, cb in s_plan:
                    c0 = cs * 128
                    w = L - c0
                    a1 = att.tile([128, L], bf16, tag="a1", name=f"a1_{h}_{cs}")
                    usl = v_u[:, h, cs:cs + 1]
                    if cs in (1, 2):
                        nc.scalar.activation(a1[:, c0:L], ps1[:, cb:cb + w],
                                             Cpy, scale=usl)
                    else:
                        nc.vector.tensor_scalar(a1[:, c0:L], ps1[:, cb:cb + w],
                                                usl, None, MUL)
                    nc.gpsimd.tensor_tensor(a1[:, c0:c0 + 128],
                                            a1[:, c0:c0 + 128], v_mi[:], MUL)
                    A1.append(a1)

                # branch 2 S matrix (independent of branch 1) -------------
                pa2 = psD.tile([128, L], f32, tag="pd", name=f"pa2_{h}")
                pb2 = psD.tile([128, L], f32, tag="pd", name=f"pb2_{h}")
                pc2 = psD.tile([128, L], f32, tag="pd", name=f"pc2_{h}")
                s2_plan = [(0, pa2, 0), (1, pb2, 0), (2, pc2, 0), (3, pc2, 256)]
                for cs, ps2, cb in s2_plan:
                    c0 = cs * 128
                    MM(ps2[:, cb:cb + L - c0], k2t[:, c0:c0 + 128],
                       q2s[:, c0:L], start=(cb == 0), stop=True,
                       skip_group_check=True)
                # packed A2: [cs0 512 | cs1 384 | cs2 256 | cs3 128] * 0.125,
                # strict-causal diag masks at offsets 0/512/896/1152
                a2p = att.tile([128, 1280], bf16, tag="a2", name=f"a2_{h}")
                A2OFF = [0, 512, 896, 1152]
                nc.vector.tensor_scalar(a2p[:, 0:512], pa2[:], 0.125, None, MUL)
                nc.scalar.activation(a2p[:, 512:896], pb2[:, 0:384],
                                     Cpy, scale=0.125)
                nc.scalar.activation(a2p[:, 896:1280], pc2[:, 0:384],
                                     Cpy, scale=0.125)
                for cs in range(NCH):
                    nc.gpsimd.tensor_tensor(a2p[:, A2OFF[cs]:A2OFF[cs] + 128],
                                            a2p[:, A2OFF[cs]:A2OFF[cs] + 128],
                                            v_ms8[:], MUL)

                # O1^T (d-major, gc-scaled via qtg); cols c0:c0+128 of the
                # accumulation are final after the cs-th matmul, so the
                # t-major E extraction pipelines chunk by chunk.  Branch 2
                # then accumulates O2^T INTO THE SAME BANK, so the combined
                # outT needs no add - just one evacuation at the end.
                po = pot.tile([64, L], f32, tag="pot", name=f"po_{h}")
                for cs in range(NCH):
                    c0 = cs * 128
                    MM(po[:, c0:L], v_v[:, h, cs, :], A1[cs][:, c0:L],
                       start=(cs == 0), stop=False,
                       skip_group_check=True)

                Et = att.tile([128, NCH, DH], bf16, tag="et", bufs=2,
                              name=f"et_{h}")
                po1Ts = sm.tile([64, NCH, 128], bf16, tag="po1Ts", bufs=2,
                                name=f"po1Ts_{h}")
                for ct in range(NCH):
                    t0 = ct * 128
                    if ct % 2 == 0:
                        nc.vector.tensor_copy(po1Ts[:, ct, :],
                                              po[:, t0:t0 + 128])
                    else:
                        nc.scalar.copy(po1Ts[:, ct, :], po[:, t0:t0 + 128])
                    ptr = psD.tile([128, DH], bf16, tag="ptr", bufs=1,
                                   name=f"ptr_{h}_{ct}")
                    nc.tensor.transpose(ptr[:], po1Ts[:, ct, :], v_idn)
                    nc.vector.tensor_tensor(Et[:, ct, :], v_vs[:, h, ct, :],
                                            ptr[:], mybir.AluOpType.subtract)

                # O2^T accumulates into po on top of gc*O1^T
                for cs in range(NCH):
                    c0 = cs * 128
                    MM(po[:, c0:L], Et[:, cs, :],
                       a2p[:, A2OFF[cs]:A2OFF[cs] + L - c0],
                       start=False, stop=(cs == NCH - 1),
                       skip_group_check=True)
                # evacuate combined outT in halves on both engines
                nc.vector.tensor_copy(outT2[rows, hp, 0:256], po[:, 0:256])
                nc.scalar.copy(outT2[rows, hp, 256:L], po[:, 256:L])

            emit_proj(0)
            emit_head(0)
            emit_head(1)
            emit_proj(1)

            # hp=0 halves of the first output-projection tiles can run as
            # soon as heads 0/1 land (psA banks are free after proj 1)
            ysb = qk.tile([128, KC, L], bf16)
            yT_r = yT.ap().rearrange("(c p) t -> p c t", p=128)
            emit_head(2)
            emit_head(3)

            for nci in range(KC):
                n0 = nci * 128
                pool = psA if nci % 2 == 0 else pot
                tg = "pbig" if nci % 2 == 0 else "pot"
                py = pool.tile([128, L], f32, tag=tg, name=f"py_{nci}")
                for hp in range(2):
                    MM(py[:], v_wcp[:, hp, n0:n0 + 128],
                       outT2[:, hp, :], start=(hp == 0), stop=(hp == 1))
                if nci % 2 == 0:
                    nc.vector.tensor_copy(ysb[:, nci, :], py[:])
                else:
                    nc.scalar.copy(ysb[:, nci, :], py[:])
                # single-tile output DMAs, alternating free queues
                eng = nc.sync if nci % 2 == 0 else nc.gpsimd
                eng.dma_start(out=yT_r[:, nci:nci + 1, :],
                              in_=ysb[:, nci:nci + 1, :])

    # this walrus build allows ONE sync wait per engine instruction; Tile's
    # final drain carries the whole vector clock, so split multi-wait
    # instructions into chains of single-wait drains for the HW build.
    if not split_waits:
        return nc
    import concourse.mybir as mybir
    nsplit = 0
    for bb in nc.m.functions[0].blocks:
        new_insts = []
        for inst in bb.instructions:
            si = getattr(inst, "sync_info", None)
            if si is not None and si.on_wait and len(si.on_wait) > 1:
                waits = list(si.on_wait)
                for j, w in enumerate(waits[:-1]):
                    d = mybir.InstDrain(
                        name=f"{inst.name}_sw{j}", engine=inst.engine,
                        ins=[], outs=[],
                        sync_info=mybir.SyncInfo(on_wait=[w], on_update=[]))
                    new_insts.append(d)
                    nsplit += 1
                si.on_wait = [waits[-1]]
            new_insts.append(inst)
        bb.instructions = new_insts
    return nc


# ----------------------------------------------------------------- host side
def _sigmoid(z):
    return 1.0 / (1.0 + np.exp(-z))


def _pack_core(core, x, Gc, U, wq_h, wk_h, wk2_h, wcp_h, bq_h, bk_h, bk2s_h,
               mi, ms8):
    b = core // 4
    h0 = (core % 4) * 4

    pk = np.zeros((128, TOT), dtype=np.float32)

    def put(name, arr):
        a, bb, dt = _REG[name]
        arr = np.asarray(arr)
        arr2 = arr.reshape(arr.shape[0], -1)
        if dt == 'b':
            a16 = arr2.astype(ml_dtypes.bfloat16).view(np.uint16)
            u32 = (a16[:, 0::2].astype(np.uint32)
                   | (a16[:, 1::2].astype(np.uint32) << 16))
            pk[:arr.shape[0], a:a + u32.shape[1]] = u32.view(np.float32)
        elif dt == 'x':
            a8 = arr2.astype(ml_dtypes.float8_e4m3).view(np.uint8)
            u32 = (a8[:, 0::4].astype(np.uint32)
                   | (a8[:, 1::4].astype(np.uint32) << 8)
                   | (a8[:, 2::4].astype(np.uint32) << 16)
                   | (a8[:, 3::4].astype(np.uint32) << 24))
            pk[:arr.shape[0], a:a + u32.shape[1]] = u32.view(np.float32)
        else:
            pk[:arr.shape[0], a:bb] = arr2

    xb = x[b]                                        # (L, D)
    xTc = xb.T.reshape(KC, 128, L).transpose(1, 0, 2)
    put("xT", xTc)
    put("xT8", xTc)
    put("wk28", wk2_h[core])
    put("wq", wq_h[core])
    put("wk", wk_h[core])
    put("wcp", wcp_h[core])

    hsl = slice(h0 * DH, (h0 + 4) * DH)
    vh = xb[:, hsl].reshape(L, 4, DH)
    put("v", vh.reshape(NCH, 128, 4, DH).transpose(1, 2, 0, 3))
    vsh = np.zeros_like(vh)
    vsh[:-1] = vh[1:]
    put("vs", vsh.reshape(NCH, 128, 4, DH).transpose(1, 2, 0, 3))

    gch = Gc[b, :, h0:h0 + 4]                        # (L, 4)
    uh = U[b, :, h0:h0 + 4]
    # gcp[r, hp, t] = Gc[t, hp*2 + r//64]: row-head layout matching QT
    gcp = np.empty((128, 2, L), dtype=np.float32)
    for hp in range(2):
        for l in range(2):
            gcp[l * 64:(l + 1) * 64, hp, :] = gch[:, hp * 2 + l]
    put("gcp", gcp)
    put("u", uh.reshape(NCH, 128, 4).transpose(1, 2, 0))
    put("gc", gch.reshape(NCH, 128, 4).transpose(1, 2, 0))
    put("bq", bq_h[core])
    put("bk", bk_h[core])
    put("bqr", 0.98 * bq_h[core])
    put("bk2s", bk2s_h[core])
    put("mi", mi)
    put("ms8", ms8)
    put("idn", np.eye(64, dtype=np.float32))
    return pk


def kernel(**inputs):
    inputs = {k: np.asarray(v, dtype=np.float32) for k, v in inputs.items()}
    x = inputs["x"]
    q1_w, q1_b = inputs["q1_w"], inputs["q1_b"]
    k1_w, k1_b = inputs["k1_w"], inputs["k1_b"]
    k2_w, k2_b = inputs["k2_w"], inputs["k2_b"]
    gw_w, gw_b = inputs["gw_w"], inputs["gw_b"]
    sw_w, sw_b = inputs["sw_w"], inputs["sw_b"]
    cp_w, cp_b = inputs["cp_w"], inputs["cp_b"]

    # host scalars: gate cumprod Gc and u = silu(K@sw)/Gc  (tiny matvecs)
    xh = x.reshape(B, L, H, DH)
    zg = np.einsum("blhd,d->blh", xh, gw_w[:, 0], optimize=True) + gw_b[0]
    G = _sigmoid(zg)
    log_cp = np.clip(np.cumsum(np.log(np.clip(G, 1e-6, None)), axis=1),
                     -30.0, 30.0)
    Gc = (np.exp(log_cp) + 1e-6).astype(np.float32)        # (B, L, H)

    Wr = (k1_w.reshape(D, H, DH) @ sw_w[:, 0]).astype(np.float32)
    Cr = (k1_b.reshape(H, DH) @ sw_w[:, 0]) + sw_b[0]
    zr = x @ Wr + Cr
    R = zr * _sigmoid(zr)
    U = (R / Gc).astype(np.float32)

    idx = np.arange(128)
    mi = (idx[:, None] <= idx[None, :]).astype(np.float32)
    ms8 = (idx[:, None] < idx[None, :]).astype(np.float32)
    c2 = np.float32(0.02 / np.sqrt(np.float32(D)))

    wq_h, wk_h, wk2_h, wcp_h, bq_h, bk_h, bk2s_h = {}, {}, {}, {}, {}, {}, {}
    for core in range(N_CORES):
        h0 = (core % 4) * 4
        hsl = slice(h0 * DH, (h0 + 4) * DH)
        wq_h[core] = q1_w[:, hsl].reshape(KC, 128, 256).transpose(1, 0, 2)
        wk_h[core] = k1_w[:, hsl].reshape(KC, 128, 256).transpose(1, 0, 2)
        wk2_h[core] = k2_w[:, hsl].reshape(KC, 128, 256).transpose(1, 0, 2)
        wcp_h[core] = cp_w[hsl, :].reshape(2, 128, D).transpose(1, 0, 2)
        bq_h[core] = q1_b[hsl].reshape(2, 128).T
        bk_h[core] = k1_b[hsl].reshape(2, 128).T
        bk2s_h[core] = (k2_b[hsl] * c2).reshape(2, 128).T

    in_maps = [
        {"inp": _pack_core(core, x, Gc, U, wq_h, wk_h, wk2_h, wcp_h,
                           bq_h, bk_h, bk2s_h, mi, ms8)}
        for core in range(N_CORES)
    ]

    _CACHE["last_in_maps"] = in_maps
    res = _run(in_maps)

    y = np.empty((B, L, D), dtype=np.float32)
    for b in range(B):
        acc = res[4 * b]["yT"].astype(np.float32)
        for c in range(4 * b + 1, 4 * b + 4):
            acc = acc + res[c]["yT"]
        y[b] = acc.T
    y += cp_b
    return y


def _run(in_maps, trace=False):
    if "nc" not in _CACHE:
        _CACHE["nc"] = _build_nc()
    from concourse.bass_utils import run_bass_kernel_spmd
    r = run_bass_kernel_spmd(_CACHE["nc"], in_maps,
                             core_ids=list(range(N_CORES)), trace=trace)
    _CACHE["last"] = r
    return r.results


# revision 40
# speedup vs baseline: 1.0395x; 1.0166x over previous
"""GatedLinearAttentionARMA on 8 Trainium2 NeuronCores (Bass/Tile).

B=2, L=512, D=1024, H=16, DH=64.

Both recurrences are linear scans of rank-1 updates, exactly equivalent to
causal quadratic attention:
  O1_t = Gc_t * sum_{s<=t} (Q_t.K_s) * u_s * V_s        (u = silu(K@sw)/Gc)
  E_t  = V_{t+1} - O1_t
  O2_t = sum_{s<t} (q2_{t-1}.k2_s) * E_s
  y    = (O1 + O2) @ cp_w + cp_b

Sharding: core c handles batch b = c//4 and heads (c%4)*4 .. +3.  Each core
emits its heads' d-major partial output yT (1024, 512); the host sums the 4
per-batch partials, transposes, and adds cp_b.

v2 device program (vs the 47.5us baseline):
  - all matmul operands shipped/evacuated as bf16 (PSUM accumulation stays
    f32): halves DMA bytes and makes <256-col matmuls 4x faster on PE.
  - inputs ride 3 parallel DMA queues (SP / Act / Pool) ordered so the first
    projection can start ~3.5us in; outputs are spread over all 3 queues.
  - activation-table warmup at t~0 on a memset tile (the 1.28us table load
    otherwise lands in the first projection evacuation).
  - PSUM evacuations balanced across Act/DVE; masks and m1 on Pool.
"""

import sys

if "/opt/trn_rl_repo" not in sys.path:
    sys.path.insert(0, "/opt/trn_rl_repo")

import numpy as np
import ml_dtypes

B, L, D, H = 2, 512, 1024, 16
DH = D // H            # 64
NCH = L // 128         # 4 time chunks
KC = D // 128          # 8 contraction chunks
N_CORES = 8

# --- packed input column layout (fp32-column units) -----------------------
# dtype 'b' = bf16 (2 elems per fp32 column), 'f' = fp32
_REG = {}
_c = 0
def _alloc(name, cols_f32, dt):
    global _c
    _REG[name] = (_c, _c + cols_f32, dt)
    _c += cols_f32
# Act queue: wq + masks + idn + scalars (needed first by PE / Pool masks)
_alloc("wq", KC * 128, 'b')         # (128, 8, 256) bf16
_alloc("mi", 64, 'b')               # (128, 128) incl causal mask
_alloc("ms8", 64, 'b')              # strict causal mask
_alloc("idn", 32, 'b')              # (64, 64) identity for PE transpose
_alloc("u", 16, 'f')                # (128, 4, 4)
_alloc("gc", 16, 'f')
_alloc("bq", 2, 'f')
_alloc("bk", 2, 'f')
_alloc("bqr", 2, 'f')               # 0.98*bq
_alloc("bk2s", 2, 'f')              # c2*k2_b
R_ACT_END = _c
_alloc("wcp", 2 * D // 2, 'b')      # (128, 2, 1024) bf16  (Act queue tail)
R_ACT2_END = _c
# SP queue: xT (split in 4), xT8 (fp8 copy for the DoubleRow k2 projection)
_alloc("xT", KC * 256, 'b')         # (128, 8, 512) bf16
_alloc("xT8", KC * 128, 'x')        # (128, 4, 2, 512) fp8e4
R_SP_END = _c
# Pool queue: wk, wk28, v, vs, gcp
_alloc("wk", KC * 128, 'b')
_alloc("wk28", KC * 64, 'x')        # (128, 4, 2, 256) fp8e4
_alloc("v", 4 * NCH * DH // 2, 'b')     # (128, 4, 4, 64) bf16
_alloc("vs", 4 * NCH * DH // 2, 'b')
_alloc("gcp", 2 * L // 2, 'b')          # (128p, 2, 512) bf16: Gc per (row-head, hp)
TOT = _c

_CACHE = {}


def _build_nc(split_waits=True):
    import concourse.bass as bass
    import concourse.mybir as mybir
    from concourse.tile import TileContext

    f32 = mybir.dt.float32
    f32r = mybir.dt.float32r
    bf16 = mybir.dt.bfloat16
    nc = bass.Bass()

    inp = nc.dram_tensor("inp", [128, TOT], f32r, kind="ExternalInput")
    yT = nc.dram_tensor("yT", [D, L], bf16, kind="ExternalOutput")

    C2 = 0.02 / float(np.sqrt(D))
    Ident = mybir.ActivationFunctionType.Identity
    Sig = mybir.ActivationFunctionType.Sigmoid
    Rel = mybir.ActivationFunctionType.Relu
    Cpy = mybir.ActivationFunctionType.Copy
    MUL = mybir.AluOpType.mult

    with TileContext(nc) as tc:
        with (
            tc.tile_pool(name="cst", bufs=1) as cst,
            tc.tile_pool(name="qk", bufs=1) as qk,
            tc.tile_pool(name="att", bufs=8) as att,
            tc.tile_pool(name="sm", bufs=4) as sm,
            tc.tile_pool(name="psA", bufs=2, space="PSUM") as psA,
            tc.tile_pool(name="psD", bufs=3, space="PSUM") as psD,
            tc.tile_pool(name="pot", bufs=2, space="PSUM") as pot,
        ):
            mega = cst.tile([128, TOT], f32r)

            # --- input DMAs: 3 parallel queues, first-needed-first --------
            # SP: xT in 4 pieces (drip-feeds the kc-interleaved projections),
            # then wcp.  Act: wq in 2 pieces, then masks+scalars, then free.
            # Pool: wk, wk2, v+vs+gcb.
            a0 = _REG["xT"][0]
            a8 = _REG["xT8"][0]
            for i in range(2):
                nc.sync.dma_start(out=mega[:, a0 + 512 * i:a0 + 512 * (i + 1)],
                                  in_=inp[:, a0 + 512 * i:a0 + 512 * (i + 1)])
            nc.sync.dma_start(out=mega[:, a8:R_SP_END], in_=inp[:, a8:R_SP_END])
            nc.sync.dma_start(out=mega[:, _REG["wcp"][0]:R_ACT2_END],
                              in_=inp[:, _REG["wcp"][0]:R_ACT2_END])
            nc.scalar.dma_start(out=mega[:, 0:128], in_=inp[:, 0:128])
            nc.scalar.dma_start(out=mega[:, 128:1024], in_=inp[:, 128:1024])
            for i in range(2, 4):
                nc.scalar.dma_start(out=mega[:, a0 + 512 * i:a0 + 512 * (i + 1)],
                                    in_=inp[:, a0 + 512 * i:a0 + 512 * (i + 1)])
            nc.scalar.dma_start(out=mega[:, 1024:R_ACT_END],
                                in_=inp[:, 1024:R_ACT_END])
            b0, b1 = _REG["wk"][0], _REG["wk28"][0]
            b2 = _REG["v"][0]
            nc.gpsimd.dma_start(out=mega[:, b0:b1], in_=inp[:, b0:b1])
            nc.gpsimd.dma_start(out=mega[:, b1:b2], in_=inp[:, b1:b2])
            nc.gpsimd.dma_start(out=mega[:, b2:TOT], in_=inp[:, b2:TOT])

            def bview(name, rows, *shape):
                a, b, dt = _REG[name]
                assert dt == 'b'
                ap = mega[0:rows, a:b].bitcast(bf16)
                if len(shape) > 1:
                    ap = ap.rearrange(
                        "p (" + " ".join(f"d{i}" for i in range(len(shape))) + ") -> p "
                        + " ".join(f"d{i}" for i in range(len(shape))),
                        **{f"d{i}": s for i, s in enumerate(shape)},
                    )
                return ap

            fp8 = mybir.dt.float8e4

            def xview(name, *shape):
                a, b, dt_ = _REG[name]
                ap = mega[:, a:b].bitcast(fp8)
                return ap.rearrange(
                    "p (" + " ".join(f"d{i}" for i in range(len(shape))) + ") -> p "
                    + " ".join(f"d{i}" for i in range(len(shape))),
                    **{f"d{i}": s for i, s in enumerate(shape)},
                )

            v_xT = bview("xT", 128, KC, L)
            v_xT8 = xview("xT8", KC // 2, 2, L)
            v_wk28 = xview("wk28", KC // 2, 2, 256)
            v_wq = bview("wq", 128, KC, 256)
            v_wk = bview("wk", 128, KC, 256)
            v_wcp = bview("wcp", 128, 2, D)
            v_v = bview("v", 128, 4, NCH, DH)
            v_vs = bview("vs", 128, 4, NCH, DH)
            v_gcp = bview("gcp", 128, 2, L)
            v_mi = bview("mi", 128, 128)
            v_ms8 = bview("ms8", 128, 128)
            v_idn = bview("idn", 64, 64)

            # --- act-table warmup at t~0 (Sigmoid loads the shared table) -
            wrm = sm.tile([1, 2], f32, tag="wrm", bufs=1)
            nc.gpsimd.memset(wrm[:], 0.25)
            nc.scalar.activation(wrm[0:1, 1:2], wrm[0:1, 0:1], Sig, scale=1.0)

            # --- PE p-state warmup: ~2us of dummy matmuls on a memset tile
            # (PE ramps 0.65->1.2->2.4GHz over 3us of continuous activity;
            # burn the ramp before the first weights arrive).
            grb = cst.tile([128, 256], bf16)
            nc.vector.memset(grb[:], 0.125)
            pdum = psD.tile([128, 256], f32, tag="ptr", bufs=1, name="pdum")
            NDUM = 9
            for i in range(NDUM):
                nc.tensor.matmul(pdum[:], grb[:, 0:128], grb[:, 0:256],
                                 start=(i == 0), stop=(i == NDUM - 1))

            # f32 copies of the per-partition scalar block (tensor_scalar and
            # activation bias operands must be plain float32)
            sc = cst.tile([128, 32], f32)    # u(16) + gc(16)
            nc.vector.tensor_copy(sc[:], mega[:, _REG["u"][0]:_REG["gc"][1]])
            scA = cst.tile([128, 8], f32)    # bq bk bqr bk2s
            nc.scalar.copy(scA[:], mega[:, _REG["bq"][0]:_REG["bk2s"][1]])
            v_u = sc[:, 0:16].rearrange("p (h c) -> p h c", h=4)
            v_gc = sc[:, 16:32].rearrange("p (h c) -> p h c", h=4)
            v_bq = scA[:, 0:2]
            v_bk = scA[:, 2:4]
            v_bqr = scA[:, 4:6]
            v_bk2s = scA[:, 6:8]

            def MM(out, lhsT, rhs, **kw):
                return nc.tensor.matmul(out, lhsT, rhs, **kw)

            # ---- projections: d-major QT/KT/k2T (2 heads per tile) ------
            QT = qk.tile([128, 2, L], bf16)
            QTG = qk.tile([128, 2, L], bf16)   # QT * Gc_t (per row-head)
            KT = qk.tile([128, 2, L], bf16)
            K2T = qk.tile([128, 2, L], bf16)
            Q2M = qk.tile([128, 2, L], bf16)   # min(Q, 0.02Q)
            Q2S = qk.tile([128, 2, L], bf16)   # Q2M shifted right by one

            def emit_proj(hp):
                # q and k matmuls interleaved kc-major so the xT chunks are
                # consumed as they arrive off the drip-fed SP queue.
                mcol = slice(hp * 128, (hp + 1) * 128)
                pq = psA.tile([128, L], f32, tag="pbig", name=f"pq_{hp}")
                pk = psA.tile([128, L], f32, tag="pbig", name=f"pk_{hp}")
                for kc in range(KC):
                    MM(pq[:], v_wq[:, kc, mcol], v_xT[:, kc, :],
                       start=(kc == 0), stop=(kc == KC - 1))
                nc.scalar.activation(QT[:, hp, :], pq[:], Ident,
                                     bias=v_bq[:, hp:hp + 1], scale=1.0)
                for kc in range(KC):
                    MM(pk[:], v_wk[:, kc, mcol], v_xT[:, kc, :],
                       start=(kc == 0), stop=(kc == KC - 1))
                nc.scalar.activation(KT[:, hp, :], pk[:], Ident,
                                     bias=v_bk[:, hp:hp + 1], scale=1.0)
                # branch-1 Q carries the gate: qtg_t = Q_t * Gc_t
                nc.vector.tensor_tensor(QTG[:, hp, :], QT[:, hp, :],
                                        v_gcp[:, hp, :], MUL)

                pk2 = psA.tile([128, L], f32, tag="pbig", name=f"pk2_{hp}")
                for kc2 in range(KC // 2):
                    MM(pk2[:], v_wk28[:, kc2, :, mcol], v_xT8[:, kc2, :, :],
                       start=(kc2 == 0), stop=(kc2 == KC // 2 - 1),
                       perf_mode=mybir.MatmulPerfMode.DoubleRow)
                nc.scalar.activation(K2T[:, hp, :], pk2[:], Sig,
                                     bias=v_bk2s[:, hp:hp + 1], scale=C2)

                # q2m = min(Q, 0.02*Q)   (true q2 * 8)
                Q02 = sm.tile([128, L], bf16, tag="q02", bufs=2,
                              name=f"q02_{hp}")
                nc.vector.tensor_scalar(Q02[:], QT[:, hp, :], 0.02, None, MUL)
                nc.vector.tensor_tensor(Q2M[:, hp, :], QT[:, hp, :], Q02[:],
                                        mybir.AluOpType.min)
                nc.vector.tensor_copy(Q2S[:, hp, 1:L], Q2M[:, hp, 0:L - 1])
                nc.vector.tensor_scalar(Q2S[:, hp, 0:1], QT[:, hp, 0:1],
                                        0.0, None, MUL)

            # ---- per-head attention ------------------------------------
            outT2 = att.tile([128, 2, L], bf16, tag="otp", bufs=1)

            def emit_head(h):
                hp, r0 = h // 2, (h % 2) * 64
                rows = slice(r0, r0 + 64)
                qt = QTG[rows, hp, :]
                kt = KT[rows, hp, :]
                k2t = K2T[rows, hp, :]
                q2s = Q2S[rows, hp, :]

                # branch 1: S^T chunks -> A1 (u-scaled, causal incl).
                # cs=2 and cs=3 share one PSUM bank (column-packed).
                pa = psD.tile([128, L], f32, tag="pd", name=f"pa1_{h}")
                pb = psD.tile([128, L], f32, tag="pd", name=f"pb1_{h}")
                pc = psD.tile([128, L], f32, tag="pd", name=f"pc1_{h}")
                s_plan = [(0, pa, 0), (1, pb, 0), (2, pc, 0), (3, pc, 256)]
                for cs, ps1, cb in s_plan:
                    c0 = cs * 128
                    MM(ps1[:, cb:cb + L - c0], kt[:, c0:c0 + 128],
                       qt[:, c0:L], start=(cb == 0), stop=True,
                       skip_group_check=True)
                A1 = []
                for cs, ps1, cb in s_plan:
                    c0 = cs * 128
                    w = L - c0
                    a1 = att.tile([128, L], bf16, tag="a1", name=f"a1_{h}_{cs}")
                    usl = v_u[:, h, cs:cs + 1]
                    if cs in (1, 2):
                        nc.scalar.activation(a1[:, c0:L], ps1[:, cb:cb + w],
                                             Cpy, scale=usl)
                    else:
                        nc.vector.tensor_scalar(a1[:, c0:L], ps1[:, cb:cb + w],
                                                usl, None, MUL)
                    nc.gpsimd.tensor_tensor(a1[:, c0:c0 + 128],
                                            a1[:, c0:c0 + 128], v_mi[:], MUL)
                    A1.append(a1)

                # branch 2 S matrix (independent of branch 1) -------------
                pa2 = psD.tile([128, L], f32, tag="pd", name=f"pa2_{h}")
                pb2 = psD.tile([128, L], f32, tag="pd", name=f"pb2_{h}")
                pc2 = psD.tile([128, L], f32, tag="pd", name=f"pc2_{h}")
                s2_plan = [(0, pa2, 0), (1, pb2, 0), (2, pc2, 0), (3, pc2, 256)]
                for cs, ps2, cb in s2_plan:
                    c0 = cs * 128
                    MM(ps2[:, cb:cb + L - c0], k2t[:, c0:c0 + 128],
                       q2s[:, c0:L], start=(cb == 0), stop=True,
                       skip_group_check=True)
                # packed A2: [cs0 512 | cs1 384 | cs2 256 | cs3 128] * 0.125,
                # strict-causal diag masks at offsets 0/512/896/1152
                a2p = att.tile([128, 1280], bf16, tag="a2", name=f"a2_{h}")
                A2OFF = [0, 512, 896, 1152]
                nc.scalar.activation(a2p[:, 0:512], pa2[:],
                                     Cpy, scale=0.125)
                nc.vector.tensor_scalar(a2p[:, 512:896], pb2[:, 0:384],
                                        0.125, None, MUL)
                nc.scalar.activation(a2p[:, 896:1280], pc2[:, 0:384],
                                     Cpy, scale=0.125)
                for cs in range(NCH):
                    nc.gpsimd.tensor_tensor(a2p[:, A2OFF[cs]:A2OFF[cs] + 128],
                                            a2p[:, A2OFF[cs]:A2OFF[cs] + 128],
                                            v_ms8[:], MUL)

                # O1^T (d-major, gc-scaled via qtg); cols c0:c0+128 of the
                # accumulation are final after the cs-th matmul, so the
                # t-major E extraction pipelines chunk by chunk.  Branch 2
                # then accumulates O2^T INTO THE SAME BANK, so the combined
                # outT needs no add - just one evacuation at the end.
                po = pot.tile([64, L], f32, tag="pot", name=f"po_{h}")
                for cs in range(NCH):
                    c0 = cs * 128
                    MM(po[:, c0:L], v_v[:, h, cs, :], A1[cs][:, c0:L],
                       start=(cs == 0), stop=False,
                       skip_group_check=True)

                Et = att.tile([128, NCH, DH], bf16, tag="et", bufs=2,
                              name=f"et_{h}")
                po1Ts = sm.tile([64, NCH, 128], bf16, tag="po1Ts", bufs=2,
                                name=f"po1Ts_{h}")
                for ct in range(NCH):
                    t0 = ct * 128
                    if ct % 2 == 0:
                        nc.vector.tensor_copy(po1Ts[:, ct, :],
                                              po[:, t0:t0 + 128])
                    else:
                        nc.scalar.copy(po1Ts[:, ct, :], po[:, t0:t0 + 128])
                    ptr = psD.tile([128, DH], bf16, tag="ptr", bufs=1,
                                   name=f"ptr_{h}_{ct}")
                    nc.tensor.transpose(ptr[:], po1Ts[:, ct, :], v_idn)
                    nc.vector.tensor_tensor(Et[:, ct, :], v_vs[:, h, ct, :],
                                            ptr[:], mybir.AluOpType.subtract)

                # O2^T accumulates into po on top of gc*O1^T
                for cs in range(NCH):
                    c0 = cs * 128
                    MM(po[:, c0:L], Et[:, cs, :],
                       a2p[:, A2OFF[cs]:A2OFF[cs] + L - c0],
                       start=False, stop=(cs == NCH - 1),
                       skip_group_check=True)
                # evacuate combined outT in halves on both engines
                nc.vector.tensor_copy(outT2[rows, hp, 0:256], po[:, 0:256])
                nc.scalar.copy(outT2[rows, hp, 256:L], po[:, 256:L])

            emit_proj(0)
            emit_head(0)
            emit_head(1)
            emit_proj(1)

            # hp=0 halves of the first output-projection tiles can run as
            # soon as heads 0/1 land (psA banks are free after proj 1)
            ysb = qk.tile([128, KC, L], bf16)
            yT_r = yT.ap().rearrange("(c p) t -> p c t", p=128)
            emit_head(2)
            emit_head(3)

            for nci in range(KC):
                n0 = nci * 128
                pool = psA if nci % 2 == 0 else pot
                tg = "pbig" if nci % 2 == 0 else "pot"
                py = pool.tile([128, L], f32, tag=tg, name=f"py_{nci}")
                for hp in range(2):
                    MM(py[:], v_wcp[:, hp, n0:n0 + 128],
                       outT2[:, hp, :], start=(hp == 0), stop=(hp == 1))
                if nci % 2 == 0:
                    nc.vector.tensor_copy(ysb[:, nci, :], py[:])
                else:
                    nc.scalar.copy(ysb[:, nci, :], py[:])
                # single-tile output DMAs, alternating free queues
                eng = nc.sync if nci % 2 == 0 else nc.gpsimd
                eng.dma_start(out=yT_r[:, nci:nci + 1, :],
                              in_=ysb[:, nci:nci + 1, :])

    # this walrus build allows ONE sync wait per engine instruction; Tile's
    # final drain carries the whole vector clock, so split multi-wait
    # instructions into chains of single-wait drains for the HW build.
    if not split_waits:
        return nc
    import concourse.mybir as mybir
    nsplit = 0
    for bb in nc.m.functions[0].blocks:
        new_insts = []
        for inst in bb.instructions:
            si = getattr(inst, "sync_info", None)
            if si is not None and si.on_wait and len(si.on_wait) > 1:
                waits = list(si.on_wait)
                for j, w in enumerate(waits[:-1]):
                    d = mybir.InstDrain(
                        name=f"{inst.name}_sw{j}", engine=inst.engine,
                        ins=[], outs=[],
                        sync_info=mybir.SyncInfo(on_wait=[w], on_update=[]))
                    new_insts.append(d)
                    nsplit += 1
                si.on_wait = [waits[-1]]
            new_insts.append(inst)
        bb.instructions = new_insts
    return nc


# ----------------------------------------------------------------- host side
def _sigmoid(z):
    return 1.0 / (1.0 + np.exp(-z))


def _pack_core(core, x, Gc, U, wq_h, wk_h, wk2_h, wcp_h, bq_h, bk_h, bk2s_h,
               mi, ms8):
    b = core // 4
    h0 = (core % 4) * 4

    pk = np.zeros((128, TOT), dtype=np.float32)

    def put(name, arr):
        a, bb, dt = _REG[name]
        arr = np.asarray(arr)
        arr2 = arr.reshape(arr.shape[0], -1)
        if dt == 'b':
            a16 = arr2.astype(ml_dtypes.bfloat16).view(np.uint16)
            u32 = (a16[:, 0::2].astype(np.uint32)
                   | (a16[:, 1::2].astype(np.uint32) << 16))
            pk[:arr.shape[0], a:a + u32.shape[1]] = u32.view(np.float32)
        elif dt == 'x':
            a8 = arr2.astype(ml_dtypes.float8_e4m3).view(np.uint8)
            u32 = (a8[:, 0::4].astype(np.uint32)
                   | (a8[:, 1::4].astype(np.uint32) << 8)
                   | (a8[:, 2::4].astype(np.uint32) << 16)
                   | (a8[:, 3::4].astype(np.uint32) << 24))
            pk[:arr.shape[0], a:a + u32.shape[1]] = u32.view(np.float32)
        else:
            pk[:arr.shape[0], a:bb] = arr2

    xb = x[b]                                        # (L, D)
    xTc = xb.T.reshape(KC, 128, L).transpose(1, 0, 2)
    put("xT", xTc)
    put("xT8", xTc)
    put("wk28", wk2_h[core])
    put("wq", wq_h[core])
    put("wk", wk_h[core])
    put("wcp", wcp_h[core])

    hsl = slice(h0 * DH, (h0 + 4) * DH)
    vh = xb[:, hsl].reshape(L, 4, DH)
    put("v", vh.reshape(NCH, 128, 4, DH).transpose(1, 2, 0, 3))
    vsh = np.zeros_like(vh)
    vsh[:-1] = vh[1:]
    put("vs", vsh.reshape(NCH, 128, 4, DH).transpose(1, 2, 0, 3))

    gch = Gc[b, :, h0:h0 + 4]                        # (L, 4)
    uh = U[b, :, h0:h0 + 4]
    # gcp[r, hp, t] = Gc[t, hp*2 + r//64]: row-head layout matching QT
    gcp = np.empty((128, 2, L), dtype=np.float32)
    for hp in range(2):
        for l in range(2):
            gcp[l * 64:(l + 1) * 64, hp, :] = gch[:, hp * 2 + l]
    put("gcp", gcp)
    put("u", uh.reshape(NCH, 128, 4).transpose(1, 2, 0))
    put("gc", gch.reshape(NCH, 128, 4).transpose(1, 2, 0))
    put("bq", bq_h[core])
    put("bk", bk_h[core])
    put("bqr", 0.98 * bq_h[core])
    put("bk2s", bk2s_h[core])
    put("mi", mi)
    put("ms8", ms8)
    put("idn", np.eye(64, dtype=np.float32))
    return pk


def kernel(**inputs):
    inputs = {k: np.asarray(v, dtype=np.float32) for k, v in inputs.items()}
    x = inputs["x"]
    q1_w, q1_b = inputs["q1_w"], inputs["q1_b"]
    k1_w, k1_b = inputs["k1_w"], inputs["k1_b"]
    k2_w, k2_b = inputs["k2_w"], inputs["k2_b"]
    gw_w, gw_b = inputs["gw_w"], inputs["gw_b"]
    sw_w, sw_b = inputs["sw_w"], inputs["sw_b"]
    cp_w, cp_b = inputs["cp_w"], inputs["cp_b"]

    # host scalars: gate cumprod Gc and u = silu(K@sw)/Gc  (tiny matvecs)
    xh = x.reshape(B, L, H, DH)
    zg = np.einsum("blhd,d->blh", xh, gw_w[:, 0], optimize=True) + gw_b[0]
    G = _sigmoid(zg)
    log_cp = np.clip(np.cumsum(np.log(np.clip(G, 1e-6, None)), axis=1),
                     -30.0, 30.0)
    Gc = (np.exp(log_cp) + 1e-6).astype(np.float32)        # (B, L, H)

    Wr = (k1_w.reshape(D, H, DH) @ sw_w[:, 0]).astype(np.float32)
    Cr = (k1_b.reshape(H, DH) @ sw_w[:, 0]) + sw_b[0]
    zr = x @ Wr + Cr
    R = zr * _sigmoid(zr)
    U = (R / Gc).astype(np.float32)

    idx = np.arange(128)
    mi = (idx[:, None] <= idx[None, :]).astype(np.float32)
    ms8 = (idx[:, None] < idx[None, :]).astype(np.float32)
    c2 = np.float32(0.02 / np.sqrt(np.float32(D)))

    wq_h, wk_h, wk2_h, wcp_h, bq_h, bk_h, bk2s_h = {}, {}, {}, {}, {}, {}, {}
    for core in range(N_CORES):
        h0 = (core % 4) * 4
        hsl = slice(h0 * DH, (h0 + 4) * DH)
        wq_h[core] = q1_w[:, hsl].reshape(KC, 128, 256).transpose(1, 0, 2)
        wk_h[core] = k1_w[:, hsl].reshape(KC, 128, 256).transpose(1, 0, 2)
        wk2_h[core] = k2_w[:, hsl].reshape(KC, 128, 256).transpose(1, 0, 2)
        wcp_h[core] = cp_w[hsl, :].reshape(2, 128, D).transpose(1, 0, 2)
        bq_h[core] = q1_b[hsl].reshape(2, 128).T
        bk_h[core] = k1_b[hsl].reshape(2, 128).T
        bk2s_h[core] = (k2_b[hsl] * c2).reshape(2, 128).T

    in_maps = [
        {"inp": _pack_core(core, x, Gc, U, wq_h, wk_h, wk2_h, wcp_h,
                           bq_h, bk_h, bk2s_h, mi, ms8)}
        for core in range(N_CORES)
    ]

    _CACHE["last_in_maps"] = in_maps
    res = _run(in_maps)

    y = np.empty((B, L, D), dtype=np.float32)
    for b in range(B):
        acc = res[4 * b]["yT"].astype(np.float32)
        for c in range(4 * b + 1, 4 * b + 4):
            acc = acc + res[c]["yT"]
        y[b] = acc.T
    y += cp_b
    return y


def _run(in_maps, trace=False):
    if "nc" not in _CACHE:
        _CACHE["nc"] = _build_nc()
    from concourse.bass_utils import run_bass_kernel_spmd
    r = run_bass_kernel_spmd(_CACHE["nc"], in_maps,
                             core_ids=list(range(N_CORES)), trace=trace)
    _CACHE["last"] = r
    return r.results
